# revision 5
# baseline (speedup 1.0000x reference)
"""HGAT (GRU + decayed attention + 2x HypergraphConv over 9 hypergraphs) on 8 trn2 cores.

Strategy:
  - Host: densify each hypergraph incidence list into dense [1152,1152]
    operators holding RAW integer counts (exact in fp8 e4m3), shipped in both
    layouts (node-major / edge-major).  B^-1 / D^-1 / bias scalings are folded
    into cheap per-column vector ops / rank-1 matmuls on device.
  - Device (SPMD, 8 cores): GRU+attention sharded over nodes (144/core),
    AllGather (bf16) the attention output, data-parallel hypergraph convs
    (core c: timestep c) with column-tiled matmul pairs (2 concurrent 64-wide
    output groups), AllGather timestep conv results + per-timestep sums
    (bf16, sums as hi/lo bf16 pair), the global-hyp conv computed redundantly
    on all cores DURING the second AllGather, final temporal attention +
    output head with a PE-broadcast of softmax weights (no DRAM bounce).
"""
import numpy as np
import ml_dtypes

N, T, H, F_IN, E = 1026, 8, 64, 5, 1026
NP = 1152            # padded N and E (9 * 128)
NCORES = 8
SL = NP // NCORES    # 144 nodes per core
NCH = NP // 128      # 9 contraction chunks
BF = ml_dtypes.bfloat16
F8 = ml_dtypes.float8_e4m3

_NC_CACHE = {}


# --------------------------------------------------------------------------
# host-side prep
# --------------------------------------------------------------------------

def _densify(idx):
    node = idx[0].astype(np.int64)
    edge = idx[1].astype(np.int64)
    Hm = np.bincount(node * NP + edge, minlength=N * NP).reshape(N, NP)
    Hp = np.zeros((NP, NP), np.float32)
    Hp[:N] = Hm.astype(np.float32)
    degn = Hp.sum(1)
    dege = Hp.sum(0)
    Dinv = np.where(degn > 0, 1.0 / degn, 0.0).astype(np.float32)
    Binv = np.where(dege > 0, 1.0 / dege, 0.0).astype(np.float32)
    Hn = np.ascontiguousarray(Hp.astype(F8))                  # [n, e] raw counts
    HTe = np.ascontiguousarray(Hp.T.astype(F8))               # [e, n] raw counts
    return Hn, HTe, degn, Dinv, Binv


def _host_prep(inp):
    f32 = np.float32
    price = np.asarray(inp["price_input"], f32)          # [N, T, F]
    hyp_T = np.asarray(inp["hyp_T"])                     # [T, 2, nnz]
    hyp = np.asarray(inp["hyp"])                         # [2, nnz]

    WihT = np.ascontiguousarray(np.asarray(inp["Wih"], f32).T)   # [5, 192]
    WhhT = np.ascontiguousarray(np.asarray(inp["Whh"], f32).T)   # [64, 192]
    bih = np.asarray(inp["bih"], f32)
    bhh = np.asarray(inp["bhh"], f32)

    shared = {
        "WihT_rz": np.ascontiguousarray(WihT[:, 0:128]),
        "WihT_n": np.ascontiguousarray(WihT[:, 128:192]),
        "WhhT_rz": np.ascontiguousarray(WhhT[:, 0:128]),
        "WhhT_n": np.ascontiguousarray(WhhT[:, 128:192]),
        "bih_rz": np.ascontiguousarray(bih[0:128, None]),
        "bih_n": np.ascontiguousarray(bih[128:192, None]),
        "bhh_rz": np.ascontiguousarray(bhh[0:128, None]),
        "bhh_n": np.ascontiguousarray(bhh[128:192, None]),
        "Win": np.asarray(inp["Win"], f32),
        "Wout": np.asarray(inp["Wout"], f32),
        "delta": np.ascontiguousarray(
            np.broadcast_to(np.arange(T - 1, -1, -1, dtype=f32), (128, T))),
        "theta1": np.asarray(inp["theta1"], BF),
        "theta2": np.asarray(inp["theta2"], BF),
        "b1_row": np.ascontiguousarray(np.asarray(inp["bias1"], f32)[None, :]),
        "b2_row": np.ascontiguousarray(np.asarray(inp["bias2"], f32)[None, :]),
        "w1T": np.ascontiguousarray(np.asarray(inp["w1"], f32).T),   # [7, 64]
        "w2T": np.ascontiguousarray(np.asarray(inp["w2"], f32).T),   # [64, 7]
        "Wl": np.asarray(inp["Wl"], f32),                            # [128, 1]
        "bl_rep": np.full((128, 1), np.asarray(inp["bl"], f32)[0], f32),
        "ones_row": np.ones((1, 128), f32),
        "identF": np.eye(128, dtype=f32),
        "identB": np.eye(128, dtype=BF),
    }

    HnG, HTeG, degG, DinvG, BinvG = _densify(hyp)
    shared["Hn_G"] = HnG
    shared["HTe_G"] = HTeG
    shared["deg_G"] = np.ascontiguousarray(degG[None, :])
    shared["Binv_G"] = np.ascontiguousarray(BinvG[None, :].astype(BF))
    shared["Dinv_G"] = np.ascontiguousarray(DinvG[None, :])
    shared["Dcol_G"] = np.ascontiguousarray(DinvG.reshape(NCH, 128).T)  # [128, 9]

    price_p = np.zeros((NP, T, F_IN), f32)
    price_p[:N] = price
    ae_p = np.zeros((NP,), f32)
    ae_p[:N] = np.asarray(inp["ae"], f32)[:, 0, 0]
    ab_p = np.zeros((NP,), f32)
    ab_p[:N] = np.asarray(inp["ab"], f32)[:, 0, 0]

    in_maps = []
    for c in range(NCORES):
        sl = slice(c * SL, (c + 1) * SL)
        m = dict(shared)
        m["x5"] = np.ascontiguousarray(
            price_p[sl].transpose(2, 1, 0).reshape(F_IN, T * SL))     # [5, (t n)]
        m["ae_col"] = np.ascontiguousarray(ae_p[sl, None])
        m["ab_col"] = np.ascontiguousarray(ab_p[sl, None])
        HnL, HTeL, degL, DinvL, BinvL = _densify(hyp_T[c])
        m["Hn_L"] = HnL
        m["HTe_L"] = HTeL
        m["deg_L"] = np.ascontiguousarray(degL[None, :])
        m["Binv_L"] = np.ascontiguousarray(BinvL[None, :].astype(BF))
        m["Dinv_L"] = np.ascontiguousarray(DinvL[None, :])
        m["Dcol_L"] = np.ascontiguousarray(DinvL.reshape(NCH, 128).T)
        in_maps.append(m)
    return in_maps


_IN_SPECS = [
    ("x5", (F_IN, NP), "f32"),
    ("ae_col", (SL, 1), "f32"), ("ab_col", (SL, 1), "f32"),
    ("WihT_rz", (F_IN, 128), "f32"), ("WihT_n", (F_IN, 64), "f32"),
    ("WhhT_rz", (64, 128), "f32"), ("WhhT_n", (64, 64), "f32"),
    ("bih_rz", (128, 1), "f32"), ("bih_n", (64, 1), "f32"),
    ("bhh_rz", (128, 1), "f32"), ("bhh_n", (64, 1), "f32"),
    ("Win", (64, 64), "f32"), ("Wout", (128, 64), "f32"),
    ("delta", (128, T), "f32"),
    ("theta1", (64, 64), "bf16"), ("theta2", (64, 64), "bf16"),
    ("b1_row", (1, 64), "f32"), ("b2_row", (1, 64), "f32"),
    ("w1T", (T - 1, 64), "f32"), ("w2T", (64, T - 1), "f32"),
    ("Wl", (128, 1), "f32"), ("bl_rep", (128, 1), "f32"),
    ("ones_row", (1, 128), "f32"),
    ("identF", (128, 128), "f32"), ("identB", (128, 128), "bf16"),
    ("Hn_L", (NP, NP), "f8"), ("HTe_L", (NP, NP), "f8"),
    ("Hn_G", (NP, NP), "f8"), ("HTe_G", (NP, NP), "f8"),
    ("deg_L", (1, NP), "f32"), ("deg_G", (1, NP), "f32"),
    ("Binv_L", (1, NP), "bf16"), ("Binv_G", (1, NP), "bf16"),
    ("Dinv_L", (1, NP), "f32"), ("Dinv_G", (1, NP), "f32"),
    ("Dcol_L", (128, NCH), "f32"), ("Dcol_G", (128, NCH), "f32"),
]


# --------------------------------------------------------------------------
# device program
# --------------------------------------------------------------------------

def build_program(tc, A, out_ap):
    """Emit the SPMD program. A: dict name -> dram AP. out_ap: [1026,1] f32."""
    import contextlib
    import concourse.bass as bass
    import concourse.mybir as mybir

    nc = tc.nc
    F32 = mybir.dt.float32
    BF16 = mybir.dt.bfloat16
    FP8 = mybir.dt.float8e4
    AF = mybir.ActivationFunctionType
    ALU = mybir.AluOpType
    AX = mybir.AxisListType
    CH3 = ((0, 512), (512, 512), (1024, 128))
    groups = [list(range(NCORES))]

    stack = contextlib.ExitStack()
    CP = stack.enter_context(tc.tile_pool(name="consts", bufs=1))
    WK = stack.enter_context(tc.tile_pool(name="work", bufs=1))
    HP = stack.enter_context(tc.tile_pool(name="hmat", bufs=1))
    DR = stack.enter_context(tc.tile_pool(name="dram", bufs=1, space="DRAM"))

    def load(pool, name, shape, dtype, src_ap):
        t = pool.tile(shape, dtype, name=name)
        nc.sync.dma_start(t[:], src_ap)
        return t

    # ---- small consts ----
    c = {}
    for nm in ("WihT_rz", "WihT_n", "WhhT_rz", "WhhT_n", "bih_rz", "bih_n",
               "bhh_rz", "bhh_n", "Win", "Wout", "delta", "theta1", "theta2",
               "b1_row", "b2_row", "w1T", "w2T", "Wl", "bl_rep", "ones_row",
               "identF", "identB", "x5", "deg_L", "deg_G", "Dcol_L", "Dcol_G"):
        spec = dict((s[0], s) for s in _IN_SPECS)[nm]
        dt_ = {"f32": F32, "bf16": BF16, "f8": FP8}[spec[2]]
        c[nm] = load(CP, f"c_{nm}", list(spec[1]), dt_, A[nm][:])
    aeA = load(CP, "aeA", [128, 1], F32, A["ae_col"][0:128])
    aeB = load(CP, "aeB", [16, 1], F32, A["ae_col"][128:144])
    abA = load(CP, "abA", [128, 1], F32, A["ab_col"][0:128])
    abB = load(CP, "abB", [16, 1], F32, A["ab_col"][128:144])

    # broadcast rows -> [64, NP] tiles (partition-broadcast via DMA)
    bcast = {}
    for nm, dt_ in (("Binv_L", BF16), ("Binv_G", BF16),
                    ("Dinv_L", F32), ("Dinv_G", F32)):
        t = CP.tile([64, NP], dt_, name=f"bc_{nm}")
        nc.sync.dma_start(t[:], A[nm][0:1, :].broadcast_to([64, NP]))
        bcast[nm] = t

    # ---- H operator tiles (fp8, raw counts) ----
    Hmats = {}
    for nm in ("Hn_L", "HTe_L", "Hn_G", "HTe_G"):
        tiles = []
        for k in range(NCH):
            tiles.append(load(HP, f"{nm}_{k}", [128, NP], FP8,
                              A[nm][k * 128:(k + 1) * 128, :]))
        Hmats[nm] = tiles

    identF64 = c["identF"][0:64, 0:64]
    identB64 = c["identB"][0:64, 0:64]

    # ---- persistent work tiles ----
    ctxT = WK.tile([64, T * SL], F32, name="ctxT")          # [h, (t n)]
    ctx_nA = WK.tile([128, T, 64], F32, name="ctx_nA")
    ctx_nB = WK.tile([16, T, 64], F32, name="ctx_nB")
    outT_full = WK.tile([64, NP], BF16, name="outT_full")   # gathered attention out
    x1T = WK.tile([64, NP], BF16, name="x1T")               # L1 out (Dinv deferred)
    x1gT = WK.tile([64, NP], BF16, name="x1gT")             # G1 out (Dinv deferred)
    pay = WK.tile([65, NP], BF16, name="pay")               # x2 + S hi/lo row
    combT2 = WK.tile([128, NP], F32, name="combT2")         # [xgT ; xx1T]

    # ======================= GRU =======================
    with tc.tile_pool(name="sb_gi", bufs=1) as SBGI:
        gi_rz = SBGI.tile([128, T * SL], F32, name="gi_rz")
        gi_n = SBGI.tile([64, T * SL], F32, name="gi_n")
        with tc.tile_pool(name="ps_gi", bufs=1, space="PSUM") as PSGI:
            gi_rz_ps = PSGI.tile([128, T * SL], F32, name="gi_rz_ps", tag="gi")
            for o, w in CH3:
                nc.tensor.matmul(gi_rz_ps[:, o:o + w], c["WihT_rz"][:],
                                 c["x5"][:, o:o + w], start=True, stop=True)
            nc.scalar.activation(gi_rz[:], gi_rz_ps[:], AF.Identity,
                                 bias=c["bih_rz"][:])
            gi_n_ps = PSGI.tile([64, T * SL], F32, name="gi_n_ps", tag="gi")
            for o, w in CH3:
                nc.tensor.matmul(gi_n_ps[:, o:o + w], c["WihT_n"][:],
                                 c["x5"][:, o:o + w], start=True, stop=True)
            nc.scalar.activation(gi_n[:], gi_n_ps[:], AF.Identity,
                                 bias=c["bih_n"][:])

        with tc.tile_pool(name="ps_gru", bufs=1, space="PSUM") as PSG, \
             tc.tile_pool(name="sb_gru", bufs=2) as SBG:
            for t in range(T):
                s = slice(t * SL, (t + 1) * SL)
                sp = slice((t - 1) * SL, t * SL)
                rz = SBG.tile([128, SL], F32, name="rz", tag="rz")
                if t == 0:
                    nc.scalar.activation(rz[:], gi_rz[:, s], AF.Sigmoid,
                                         bias=c["bhh_rz"][:])
                    wn = SBG.tile([64, SL], F32, name="wn", tag="wn")
                    nc.vector.tensor_scalar(wn[:], rz[0:64, :], c["bhh_n"][:],
                                            None, ALU.mult)
                    un = SBG.tile([64, SL], F32, name="un", tag="un")
                    nc.vector.tensor_tensor(un[:], gi_n[:, s], wn[:], ALU.add)
                    nt = SBG.tile([64, SL], F32, name="nt", tag="nt")
                    nc.scalar.activation(nt[:], un[:], AF.Tanh)
                    z0 = SBG.tile([64, SL], F32, name="z0", tag="z0")
                    nc.scalar.activation(z0[:], rz[64:128, :], AF.Copy)
                    mt = SBG.tile([64, SL], F32, name="mt", tag="mt")
                    nc.vector.tensor_tensor(mt[:], nt[:], z0[:], ALU.mult)
                    nc.vector.tensor_tensor(ctxT[:, s], nt[:], mt[:], ALU.subtract)
                else:
                    gh_rz = PSG.tile([128, SL], F32, name="gh_rz", tag="gh_rz")
                    nc.tensor.matmul(gh_rz[:], c["WhhT_rz"][:], ctxT[:, sp],
                                     start=True, stop=True)
                    gh_n = PSG.tile([64, SL], F32, name="gh_n", tag="gh_n")
                    nc.tensor.matmul(gh_n[:], c["WhhT_n"][:], ctxT[:, sp],
                                     start=True, stop=True)
                    urz = SBG.tile([128, SL], F32, name="urz", tag="urz")
                    nc.vector.tensor_tensor(urz[:], gi_rz[:, s], gh_rz[:], ALU.add)
                    nc.scalar.activation(rz[:], urz[:], AF.Sigmoid,
                                         bias=c["bhh_rz"][:])
                    wn = SBG.tile([64, SL], F32, name="wn", tag="wn")
                    nc.vector.scalar_tensor_tensor(wn[:], gh_n[:], c["bhh_n"][:],
                                                   rz[0:64, :], ALU.add, ALU.mult)
                    un = SBG.tile([64, SL], F32, name="un", tag="un")
                    nc.vector.tensor_tensor(un[:], gi_n[:, s], wn[:], ALU.add)
                    nt = SBG.tile([64, SL], F32, name="nt", tag="nt")
                    nc.scalar.activation(nt[:], un[:], AF.Tanh)
                    z0 = SBG.tile([64, SL], F32, name="z0", tag="z0")
                    nc.scalar.activation(z0[:], rz[64:128, :], AF.Copy)
                    dt_ = SBG.tile([64, SL], F32, name="dt_", tag="dt_")
                    nc.vector.tensor_tensor(dt_[:], ctxT[:, sp], nt[:], ALU.subtract)
                    mt = SBG.tile([64, SL], F32, name="mt", tag="mt")
                    nc.vector.tensor_tensor(mt[:], dt_[:], z0[:], ALU.mult)
                    nc.vector.tensor_tensor(ctxT[:, s], mt[:], nt[:], ALU.add)
                # pipelined transpose of this step's h into node-major ctx
                trA = PSG.tile([128, 64], F32, name="trA", tag="trA")
                nc.tensor.transpose(trA[:], ctxT[:, t * SL:t * SL + 128], identF64)
                nc.scalar.activation(ctx_nA[:, t, :], trA[:], AF.Copy)
                trB = PSG.tile([16, 64], F32, name="trB", tag="trB")
                nc.tensor.transpose(trB[:], ctxT[:, t * SL + 128:(t + 1) * SL],
                                    identF64)
                nc.scalar.activation(ctx_nB[:, t, :], trB[:], AF.Copy)

        # ======================= attention =======================
        with tc.tile_pool(name="ps_att", bufs=1, space="PSUM") as PSA, \
             tc.tile_pool(name="sb_att", bufs=1) as SBA:
            lastT = ctxT[:, 7 * SL:8 * SL]
            qT_ps = PSA.tile([64, SL], F32, name="qT_ps", tag="qT")
            nc.tensor.matmul(qT_ps[:], c["Win"][:], lastT, start=True, stop=True)
            combT = SBA.tile([128, SL], F32, name="combT")
            nc.scalar.activation(combT[64:128, :], qT_ps[:], AF.Copy)

            for nm, np_, ctx_n, ae_t, ab_t, csl in (
                    ("A", 128, ctx_nA, aeA, abA, slice(0, 128)),
                    ("B", 16, ctx_nB, aeB, abB, slice(128, SL))):
                q_ps = PSA.tile([np_, 64], F32, name=f"q_ps{nm}", tag=f"q{nm}")
                nc.tensor.matmul(q_ps[:], lastT[:, csl], c["Win"][:],
                                 start=True, stop=True)
                q_s = SBA.tile([np_, 64], F32, name=f"q_s{nm}")
                nc.scalar.activation(q_s[:], q_ps[:], AF.Copy)
                prod = SBA.tile([np_, T, 64], F32, name=f"prod{nm}")
                nc.vector.tensor_tensor(
                    prod[:], ctx_n[:],
                    q_s[:].unsqueeze(1).broadcast_to([np_, T, 64]), ALU.mult)
                sc = SBA.tile([np_, T], F32, name=f"sc{nm}")
                nc.vector.tensor_reduce(sc[:], prod[:], AX.X, ALU.add)
                nm_t = SBA.tile([np_, 1], F32, name=f"nm_t{nm}")
                nc.vector.tensor_reduce(nm_t[:], sc[:], AX.X, ALU.max, negate=True)
                ex = SBA.tile([np_, T], F32, name=f"ex{nm}")
                nc.scalar.activation(ex[:], sc[:], AF.Exp, bias=nm_t[:])
                den = SBA.tile([np_, 1], F32, name=f"den{nm}")
                nc.vector.tensor_reduce(den[:], ex[:], AX.X, ALU.add)
                rcp = SBA.tile([np_, 1], F32, name=f"rcp{nm}")
                nc.vector.reciprocal(rcp[:], den[:])
                wA = SBA.tile([np_, T], F32, name=f"wA{nm}")
                nc.vector.tensor_scalar(wA[:], ex[:], rcp[:], None, ALU.mult)
                nab = SBA.tile([np_, 1], F32, name=f"nab{nm}")
                nc.vector.tensor_scalar(nab[:], ab_t[:], -1.0, None, ALU.mult)
                bt = SBA.tile([np_, T], F32, name=f"bt{nm}")
                nc.scalar.activation(bt[:], c["delta"][0:np_, :], AF.Exp,
                                     scale=nab[:])
                P_t = SBA.tile([np_, T, 64], F32, name=f"P_t{nm}")
                nc.vector.tensor_tensor(
                    P_t[:], ctx_n[:],
                    wA[:].unsqueeze(2).broadcast_to([np_, T, 64]), ALU.mult)
                G_t = SBA.tile([np_, T, 64], F32, name=f"G_t{nm}")
                nc.vector.tensor_tensor(
                    G_t[:], P_t[:],
                    bt[:].unsqueeze(2).broadcast_to([np_, T, 64]), ALU.mult)
                t2_t = SBA.tile([np_, T, 64], F32, name=f"t2_t{nm}")
                nc.scalar.activation(t2_t[:], G_t[:], AF.Relu, scale=ae_t[:])
                sm = SBA.tile([np_, T, 64], F32, name=f"sm{nm}")
                nc.vector.tensor_tensor(sm[:], P_t[:], t2_t[:], ALU.add)
                mixs = SBA.tile([np_, 64], F32, name=f"mixs{nm}")
                nc.vector.tensor_reduce(
                    mixs[:], sm[:].rearrange("p t h -> p h t"), AX.X, ALU.add)
                # transpose mixs into combT rows 0:64
                mtr = PSA.tile([64, np_], F32, name=f"mtr{nm}", tag=f"mtr{nm}")
                nc.tensor.transpose(mtr[:], mixs[:], c["identF"][0:np_, 0:np_])
                nc.scalar.activation(combT[0:64, csl], mtr[:], AF.Copy)

            outT_ps = PSA.tile([64, SL], F32, name="outT_ps", tag="outT")
            nc.tensor.matmul(outT_ps[:], c["Wout"][:], combT[:],
                             start=True, stop=True)
            outT_slice = SBA.tile([64, SL], BF16, name="outT_slice")
            nc.scalar.activation(outT_slice[:], outT_ps[:], AF.Tanh)

            # ---- collective 1: allgather attention output (bf16) ----
            cc1_in = DR.tile([64, SL], BF16, name="cc1_in")
            cc1_out = DR.tile([NCORES, 64, SL], BF16, name="cc1_out",
                              addr_space="Shared")
            nc.sync.dma_start(cc1_in[:], outT_slice[:])
            nc.gpsimd.collective_compute(
                "AllGather", ALU.bypass, replica_groups=groups,
                ins=[cc1_in[:].opt()], outs=[cc1_out[:].opt()])
            nc.sync.dma_start(
                outT_full[:].rearrange("p (c n) -> p c n", c=NCORES),
                cc1_out[:].rearrange("c p n -> p c n"))

    # ======================= hypergraph convs =======================
    PSX = stack.enter_context(tc.tile_pool(name="ps_xp", bufs=1, space="PSUM"))
    PAcc = stack.enter_context(tc.tile_pool(name="ps_acc", bufs=1, space="PSUM"))
    SBC = stack.enter_context(tc.tile_pool(name="sb_conv", bufs=2))

    EVEN = [k for k in range(NCH) if k % 2 == 0]
    ODD = [k for k in range(NCH) if k % 2 == 1]

    def conv_block(xT_in, theta_t, b_row, deg_row, Hn_ts, HTe_ts, Binv_bc,
                   Dcol, tag, out_mode, out_dst, Dinv_bc=None, S_col=None):
        """One HypergraphConv layer, feature-major, col-tiled matmul pairs.

        xT_in: [64, NP] bf16 input features (transposed).
        Dcol: None (xT_in holds true values) or [128, NCH] f32 tile of D^-1
              columns (xT_in holds un-scaled values from a previous layer).
        out_mode: 'inter' -> out_dst [64, NP] bf16 gets leaky(raw + b*deg)
                  'final' -> out_dst [64, NP] gets Dinv * leaky(raw + b*deg),
                             optional accumulated row-sum into S_col.
        """
        # xp = theta^T @ x  (per node chunk), fold Dinv of previous layer
        xp_ps = PSX.tile([128, NCH * 64], F32, name=f"xp_{tag}", tag="xp")
        for k in range(NCH):
            nc.tensor.matmul(xp_ps[:, k * 64:(k + 1) * 64],
                             xT_in[:, k * 128:(k + 1) * 128], theta_t[:],
                             start=True, stop=True)
        xpbf = SBC.tile([128, NCH, 64], BF16, name=f"xpbf_{tag}", tag="xpbf")
        if Dcol is None:
            nc.scalar.activation(
                xpbf[:], xp_ps[:].rearrange("p (k h) -> p k h", k=NCH), AF.Copy)
        else:
            for k in range(NCH):
                nc.vector.tensor_scalar(xpbf[:, k, :],
                                        xp_ps[:, k * 64:(k + 1) * 64],
                                        Dcol[:, k:k + 1], None, ALU.mult)

        # stage 1: e^T(raw) = xp^T @ Hn, col-tiled even/odd chunk pairs
        eb_ps = PAcc.tile([128, NP], F32, name=f"ebT_{tag}", tag="acc")
        for i in range(len(EVEN)):
            for o, w in CH3:
                k = EVEN[i]
                nc.tensor.matmul(eb_ps[0:64, o:o + w], xpbf[:, k, :],
                                 Hn_ts[k][:, o:o + w],
                                 start=(k == EVEN[0]), stop=(k == EVEN[-1]))
                if i < len(ODD):
                    k = ODD[i]
                    nc.tensor.matmul(eb_ps[64:128, o:o + w], xpbf[:, k, :],
                                     Hn_ts[k][:, o:o + w],
                                     start=(k == ODD[0]), stop=(k == ODD[-1]))
        # combine halves + fold B^-1 (column scale)
        e_top = SBC.tile([64, NP], BF16, name=f"etop_{tag}", tag="etop")
        nc.scalar.activation(e_top[:], eb_ps[0:64, :], AF.Copy)
        e_sum = SBC.tile([64, NP], BF16, name=f"esum_{tag}", tag="esum")
        nc.vector.tensor_tensor(e_sum[:], e_top[:], eb_ps[64:128, :], ALU.add)
        ebTbf = SBC.tile([64, NP], BF16, name=f"ebTbf_{tag}", tag="ebTbf")
        nc.vector.tensor_tensor(ebTbf[:], e_sum[:], Binv_bc[:], ALU.mult)

        # transpose e to edge-major chunks
        tr_ps = PSX.tile([128, NCH * 64], BF16, name=f"tr_{tag}", tag="xp")
        for k in range(NCH):
            nc.tensor.transpose(tr_ps[:, k * 64:(k + 1) * 64],
                                ebTbf[:, k * 128:(k + 1) * 128], identB64)
        ebbf = SBC.tile([128, NCH, 64], BF16, name=f"ebbf_{tag}", tag="ebbf")
        nc.scalar.activation(
            ebbf[:], tr_ps[:].rearrange("p (k h) -> p k h", k=NCH), AF.Copy)

        # stage 2: out^T(raw) = e^T @ HTe + b (x) deg, col-tiled
        oT_ps = PAcc.tile([128, NP], F32, name=f"oT_{tag}", tag="acc")
        for o, w in CH3:
            nc.tensor.matmul(oT_ps[0:64, o:o + w], b_row[:],
                             deg_row[:, o:o + w], start=True, stop=False)
        for i in range(len(EVEN)):
            for o, w in CH3:
                k = EVEN[i]
                nc.tensor.matmul(oT_ps[0:64, o:o + w], ebbf[:, k, :],
                                 HTe_ts[k][:, o:o + w],
                                 start=False, stop=(k == EVEN[-1]))
                if i < len(ODD):
                    k = ODD[i]
                    nc.tensor.matmul(oT_ps[64:128, o:o + w], ebbf[:, k, :],
                                     HTe_ts[k][:, o:o + w],
                                     start=(k == ODD[0]), stop=(k == ODD[-1]))
        o_top = SBC.tile([64, NP], BF16, name=f"otop_{tag}", tag="otop")
        nc.scalar.activation(o_top[:], oT_ps[0:64, :], AF.Copy)
        u = SBC.tile([64, NP], F32, name=f"u_{tag}", tag="u")
        nc.vector.tensor_tensor(u[:], o_top[:], oT_ps[64:128, :], ALU.add)
        l1 = SBC.tile([64, NP], F32, name=f"l1_{tag}", tag="lk1")
        nc.vector.tensor_scalar(l1[:], u[:], 0.2, None, ALU.mult)
        if out_mode == "inter":
            nc.vector.tensor_tensor(out_dst, u[:], l1[:], ALU.max)
        else:
            m = SBC.tile([64, NP], F32, name=f"m_{tag}", tag="m")
            nc.vector.tensor_tensor(m[:], u[:], l1[:], ALU.max)
            if S_col is not None:
                nc.vector.scalar_tensor_tensor(out_dst, m[:], 1.0, Dinv_bc[:],
                                               ALU.mult, ALU.mult,
                                               accum_out=S_col)
            else:
                nc.vector.tensor_tensor(out_dst, m[:], Dinv_bc[:], ALU.mult)

    S_col = SBC.tile([64, 1], F32, name="S_col")

    # local (timestep) convs: layer 1 then layer 2 -> pay
    conv_block(outT_full[:], c["theta1"], c["b1_row"], c["deg_L"],
               Hmats["Hn_L"], Hmats["HTe_L"], bcast["Binv_L"], None,
               "L1", "inter", x1T[:])
    conv_block(x1T[:], c["theta2"], c["b2_row"], c["deg_L"],
               Hmats["Hn_L"], Hmats["HTe_L"], bcast["Binv_L"], c["Dcol_L"],
               "L2", "final", pay[0:64, :], Dinv_bc=bcast["Dinv_L"],
               S_col=S_col)

    # S scalar -> hi/lo bf16 in pay row 64
    nc.vector.memset(pay[64:65, :], 0.0)
    S_tr = PSX.tile([1, 64], F32, name="S_tr", tag="str")
    nc.tensor.transpose(S_tr[:], S_col[:], identF64)
    S_val = SBC.tile([1, 1], F32, name="S_val")
    nc.vector.tensor_reduce(S_val[:], S_tr[:], AX.X, ALU.add)
    S_hi = SBC.tile([1, 1], BF16, name="S_hi")
    nc.vector.tensor_copy(S_hi[:], S_val[:])
    nc.vector.tensor_copy(pay[64:65, 0:1], S_hi[:])
    nc.vector.tensor_tensor(pay[64:65, 1:2], S_val[:], S_hi[:], ALU.subtract)

    # ---- collective 2: allgather conv results + sums (bf16) ----
    cc2_in = DR.tile([65, NP], BF16, name="cc2_in")
    cc2_out = DR.tile([NCORES, 65, NP], BF16, name="cc2_out",
                      addr_space="Shared")
    nc.sync.dma_start(cc2_in[:], pay[:])
    nc.gpsimd.collective_compute(
        "AllGather", ALU.bypass, replica_groups=groups,
        ins=[cc2_in[:].opt()], outs=[cc2_out[:].opt()])

    # global conv (overlaps the collective; result into combT2 rows 0:64)
    conv_block(outT_full[:], c["theta1"], c["b1_row"], c["deg_G"],
               Hmats["Hn_G"], Hmats["HTe_G"], bcast["Binv_G"], None,
               "G1", "inter", x1gT[:])
    conv_block(x1gT[:], c["theta2"], c["b2_row"], c["deg_G"],
               Hmats["Hn_G"], Hmats["HTe_G"], bcast["Binv_G"], c["Dcol_G"],
               "G2", "final", combT2[0:64, :], Dinv_bc=bcast["Dinv_G"])

    # ======================= final stage =======================
    with tc.tile_pool(name="sb_fin", bufs=1) as SBF:
        # temporal attention weights from the gathered S values
        Sg0 = SBF.tile([T - 1, 2], BF16, name="Sg0")
        nc.sync.dma_start(Sg0[:], cc2_out[0:T - 1, 64, 0:2])
        Sg1 = SBF.tile([T - 1, 2], BF16, name="Sg1")
        nc.sync.dma_start(Sg1[:], cc2_out[1:T, 64, 0:2])
        Ss0 = SBF.tile([T - 1, 1], F32, name="Ss0")
        nc.vector.tensor_reduce(Ss0[:], Sg0[:], AX.X, ALU.add)
        Ss1 = SBF.tile([T - 1, 1], F32, name="Ss1")
        nc.vector.tensor_reduce(Ss1[:], Sg1[:], AX.X, ALU.add)
        zv = SBF.tile([T - 1, 1], F32, name="zv")
        nc.vector.tensor_tensor(zv[:], Ss1[:], Ss0[:], ALU.subtract)
        y_ps = PSX.tile([64, 1], F32, name="y_ps", tag="str")
        nc.tensor.matmul(y_ps[:], c["w1T"][:], zv[:], start=True, stop=True)
        y1 = SBF.tile([64, 1], F32, name="y1")
        nc.vector.tensor_scalar(y1[:], y_ps[:], 0.2, None, ALU.mult)
        y_s = SBF.tile([64, 1], F32, name="y_s")
        nc.vector.tensor_tensor(y_s[:], y_ps[:], y1[:], ALU.max)
        wat_ps = PSX.tile([T - 1, 1], F32, name="wat_ps", tag="str")
        nc.tensor.matmul(wat_ps[:], c["w2T"][:], y_s[:], start=True, stop=True)
        wat_s = SBF.tile([T - 1, 1], F32, name="wat_s")
        nc.vector.tensor_copy(wat_s[:], wat_ps[:])
        watT_ps = PSX.tile([1, T - 1], F32, name="watT_ps", tag="str")
        nc.tensor.transpose(watT_ps[:], wat_s[:], c["identF"][0:7, 0:7])
        nmw = SBF.tile([1, 1], F32, name="nmw")
        nc.vector.tensor_reduce(nmw[:], watT_ps[:], AX.X, ALU.max, negate=True)
        den = SBF.tile([1, 1], F32, name="den")
        exw = SBF.tile([1, T - 1], F32, name="exw")
        nc.scalar.activation(exw[:], watT_ps[:], AF.Exp, bias=nmw[:],
                             accum_out=den[:])
        rw = SBF.tile([1, 1], F32, name="rw")
        nc.vector.reciprocal(rw[:], den[:])
        wsm = SBF.tile([1, T - 1], F32, name="wsm")
        nc.vector.tensor_scalar(wsm[:], exw[:], rw[:], None, ALU.mult)
        # broadcast wsm across 128 partitions via a K=1 matmul
        wbc_ps = PSX.tile([128, T - 1], F32, name="wbc_ps", tag="str")
        nc.tensor.matmul(wbc_ps[:], c["ones_row"][:], wsm[:],
                         start=True, stop=True)
        wbc = SBF.tile([128, T - 1], F32, name="wbc")
        nc.vector.tensor_copy(wbc[:], wbc_ps[:])

        # xx1 = w0*(x2[1]-x2[0]) + w2*(x2[3]-x2[2])
        x2 = []
        for t_ in range(4):
            xt_ = SBF.tile([64, NP], BF16, name=f"x2_{t_}")
            nc.sync.dma_start(xt_[:], cc2_out[t_, 0:64, :])
            x2.append(xt_)
        d0 = SBF.tile([64, NP], BF16, name="d0")
        nc.vector.tensor_tensor(d0[:], x2[1][:], x2[0][:], ALU.subtract)
        d2 = SBF.tile([64, NP], BF16, name="d2")
        nc.vector.tensor_tensor(d2[:], x2[3][:], x2[2][:], ALU.subtract)
        m0 = SBF.tile([64, NP], F32, name="m0")
        nc.vector.tensor_scalar(m0[:], d0[:], wbc[0:64, 0:1], None, ALU.mult)
        nc.vector.scalar_tensor_tensor(combT2[64:128, :], d2[:],
                                       wbc[0:64, 2:3], m0[:],
                                       ALU.mult, ALU.add)
        # output head: res[n] = leaky(Wl . comb[:, n] + bl)
        res_ps = PSX.tile([128, NCH], F32, name="res_ps", tag="str")
        for k in range(NCH):
            nc.tensor.matmul(res_ps[:, k:k + 1],
                             combT2[:, k * 128:(k + 1) * 128], c["Wl"][:],
                             start=True, stop=True)
        r1 = SBF.tile([128, NCH], F32, name="r1")
        nc.vector.tensor_scalar(r1[:], res_ps[:], c["bl_rep"][:], 0.2,
                                ALU.add, ALU.mult)
        r2 = SBF.tile([128, NCH], F32, name="r2")
        nc.vector.tensor_scalar(r2[:], res_ps[:], c["bl_rep"][:], None,
                                ALU.add)
        res_s = SBF.tile([128, NCH], F32, name="res_s")
        nc.vector.tensor_tensor(res_s[:], r2[:], r1[:], ALU.max)
        nc.sync.dma_start(
            out_ap[0:1024, 0:1].rearrange("(k p) o -> p k o", p=128),
            res_s[:, 0:8].unsqueeze(2))
        nc.sync.dma_start(out_ap[1024:1026, 0:1], res_s[0:2, 8:9])

    stack.close()


# --------------------------------------------------------------------------
# entry points
# --------------------------------------------------------------------------

def _make_nc():
    if "nc" in _NC_CACHE:
        return _NC_CACHE["nc"]
    import concourse.bacc as bacc
    import concourse.mybir as mybir
    from concourse import tile

    nc = bacc.Bacc("TRN2", target_bir_lowering=False, debug=False,
                   enable_asserts=True, num_devices=NCORES)
    A = {}
    dtmap = {"f32": mybir.dt.float32, "bf16": mybir.dt.bfloat16,
             "f8": mybir.dt.float8e4}
    for nm, shape, dt_ in _IN_SPECS:
        A[nm] = nc.dram_tensor(
            nm, list(shape), dtmap[dt_], kind="ExternalInput").ap()
    out_h = nc.dram_tensor("out", [N, 1], mybir.dt.float32,
                           kind="ExternalOutput")
    with tile.TileContext(nc) as tc:
        build_program(tc, A, out_h.ap())
    nc.compile()
    _NC_CACHE["nc"] = nc
    return nc


def kernel(**inputs):
    from concourse.bass_utils import run_bass_kernel_spmd
    nc = _make_nc()
    in_maps = _host_prep(inputs)
    res = run_bass_kernel_spmd(nc, in_maps, list(range(NCORES)))
    return np.asarray(res.results[0]["out"])


# revision 21
# speedup vs baseline: 1.2389x; 1.2389x over previous
"""HGAT (GRU + decayed attention + 2x HypergraphConv over 9 hypergraphs) on 8 trn2 cores.

Strategy:
  - Host: densify each hypergraph incidence list into dense [1152,1152]
    operators holding RAW integer counts (exact in bf16), shipped in both
    layouts (node-major / edge-major).  B^-1 / D^-1 / bias scalings are folded
    into per-column vector ops / rank-1 matmuls on device.  The attention
    decay factors exp(-ab*delta) are precomputed on host.
  - Device (SPMD, 8 cores): GRU+attention sharded over nodes (144/core) with
    gates accumulated directly in PSUM (bf16 matmuls), AllGather (bf16) the
    attention output, data-parallel hypergraph convs (core c: timestep c)
    with column-tiled matmul pairs (2 concurrent 64-wide output groups),
    AllGather timestep conv results + per-timestep sums (f32), the
    global-hyp conv computed redundantly on all cores DURING the second
    AllGather, final temporal attention + output head with a PE-broadcast of
    softmax weights (no DRAM bounce).
"""
import numpy as np
import ml_dtypes

N, T, H, F_IN, E = 1026, 8, 64, 5, 1026
NP = 1152            # padded N and E (9 * 128)
NCORES = 8
SL = NP // NCORES    # 144 nodes per core
NCH = NP // 128      # 9 contraction chunks
BF = ml_dtypes.bfloat16

_NC_CACHE = {}


# --------------------------------------------------------------------------
# host-side prep
# --------------------------------------------------------------------------

def _densify(idx):
    node = idx[0].astype(np.int64)
    edge = idx[1].astype(np.int64)
    Hm = np.bincount(node * NP + edge, minlength=N * NP).reshape(N, NP)
    Hp = np.zeros((NP, NP), np.float32)
    Hp[:N] = Hm.astype(np.float32)
    degn = Hp.sum(1)
    dege = Hp.sum(0)
    Dinv = np.where(degn > 0, 1.0 / degn, 0.0).astype(np.float32)
    Binv = np.where(dege > 0, 1.0 / dege, 0.0).astype(np.float32)
    Hn = np.ascontiguousarray(Hp.astype(BF))                  # [n, e] raw counts
    HTe = np.ascontiguousarray(Hp.T.astype(BF))               # [e, n] raw counts
    return Hn, HTe, degn, Dinv, Binv


def _host_prep(inp):
    f32 = np.float32
    price = np.asarray(inp["price_input"], f32)          # [N, T, F]
    hyp_T = np.asarray(inp["hyp_T"])                     # [T, 2, nnz]
    hyp = np.asarray(inp["hyp"])                         # [2, nnz]

    WihT = np.ascontiguousarray(np.asarray(inp["Wih"], f32).T)   # [5, 192]
    WhhT = np.ascontiguousarray(np.asarray(inp["Whh"], f32).T)   # [64, 192]
    bih = np.asarray(inp["bih"], f32)
    bhh = np.asarray(inp["bhh"], f32)

    shared = {
        "WihT_rz": np.ascontiguousarray(WihT[:, 0:128]).astype(BF),
        "WihT_n": np.ascontiguousarray(WihT[:, 128:192]).astype(BF),
        "WhhT_rz": np.ascontiguousarray(WhhT[:, 0:128]).astype(BF),
        "WhhT_n": np.ascontiguousarray(WhhT[:, 128:192]).astype(BF),
        "b_rzsum": np.ascontiguousarray((bih[0:128] + bhh[0:128])[:, None]),
        "bih_n": np.ascontiguousarray(bih[128:192, None]),
        "bhh_n": np.ascontiguousarray(bhh[128:192, None]),
        "Win": np.asarray(inp["Win"], BF),
        "Wout": np.asarray(inp["Wout"], BF),
        "theta1": np.asarray(inp["theta1"], BF),
        "theta2": np.asarray(inp["theta2"], BF),
        "b1_col": np.ascontiguousarray(np.asarray(inp["bias1"], f32)[:, None]),
        "b2_col": np.ascontiguousarray(np.asarray(inp["bias2"], f32)[:, None]),
        "w1T": np.ascontiguousarray(np.asarray(inp["w1"], f32).T),   # [7, 64]
        "w2T": np.ascontiguousarray(np.asarray(inp["w2"], f32).T),   # [64, 7]
        "Wl": np.asarray(inp["Wl"], f32),                            # [128, 1]
        "bl_rep": np.full((128, 1), np.asarray(inp["bl"], f32)[0], f32),
        "ones_row": np.ones((1, 128), f32),
        "identF": np.eye(128, dtype=f32),
        "identB": np.eye(128, dtype=BF),
    }

    HnG, HTeG, degG, DinvG, BinvG = _densify(hyp)
    shared["Hn_G"] = HnG
    shared["HTe_G"] = HTeG
    shared["Binv_G"] = np.ascontiguousarray(BinvG[None, :])
    shared["Dinv_G"] = np.ascontiguousarray(DinvG[None, :])

    price_p = np.zeros((NP, T, F_IN), f32)
    price_p[:N] = price
    ae_p = np.zeros((NP,), f32)
    ae_p[:N] = np.asarray(inp["ae"], f32)[:, 0, 0]
    ab_p = np.zeros((NP,), f32)
    ab_p[:N] = np.asarray(inp["ab"], f32)[:, 0, 0]
    delta = np.arange(T - 1, -1, -1, dtype=f32)
    bt_full = np.exp(-ab_p[:, None] * delta[None, :])    # [NP, T]

    in_maps = []
    for c in range(NCORES):
        sl = slice(c * SL, (c + 1) * SL)
        m = dict(shared)
        m["x5"] = np.ascontiguousarray(
            price_p[sl].transpose(2, 1, 0).reshape(F_IN, T * SL)).astype(BF)
        m["ae_col"] = np.ascontiguousarray(ae_p[sl, None])
        m["bt_sl"] = np.ascontiguousarray(bt_full[sl])
        HnL, HTeL, degL, DinvL, BinvL = _densify(hyp_T[c])
        m["Hn_L"] = HnL
        m["HTe_L"] = HTeL
        m["Binv_L"] = np.ascontiguousarray(BinvL[None, :])
        m["Dinv_L"] = np.ascontiguousarray(DinvL[None, :])
        in_maps.append(m)
    return in_maps


_IN_SPECS = [
    ("x5", (F_IN, NP), "bf16"),
    ("WihT_rz", (F_IN, 128), "bf16"), ("WihT_n", (F_IN, 64), "bf16"),
    ("WhhT_rz", (64, 128), "bf16"), ("WhhT_n", (64, 64), "bf16"),
    ("b_rzsum", (128, 1), "f32"),
    ("bih_n", (64, 1), "f32"), ("bhh_n", (64, 1), "f32"),
    ("identB", (128, 128), "bf16"),
    ("Hn_L", (NP, NP), "bf16"), ("HTe_L", (NP, NP), "bf16"),
    ("Hn_G", (NP, NP), "bf16"), ("HTe_G", (NP, NP), "bf16"),
    ("Win", (64, 64), "bf16"), ("Wout", (128, 64), "bf16"),
    ("ae_col", (SL, 1), "f32"), ("bt_sl", (SL, T), "f32"),
    ("identF", (128, 128), "f32"),
    ("theta1", (64, 64), "bf16"), ("theta2", (64, 64), "bf16"),
    ("b1_col", (64, 1), "f32"), ("b2_col", (64, 1), "f32"),
    ("w1T", (T - 1, 64), "f32"), ("w2T", (64, T - 1), "f32"),
    ("Wl", (128, 1), "f32"), ("bl_rep", (128, 1), "f32"),
    ("ones_row", (1, 128), "f32"),
    ("Binv_L", (1, NP), "f32"), ("Binv_G", (1, NP), "f32"),
    ("Dinv_L", (1, NP), "f32"), ("Dinv_G", (1, NP), "f32"),
]

# DMA issue order: GRU-critical first, then H operators, then attention
# consts, then conv/final consts (single in-order DMA queue).
_LOAD_ORDER = [
    "x5", "WihT_rz", "WihT_n", "WhhT_rz", "WhhT_n", "b_rzsum", "bih_n",
    "bhh_n", "identB",
    None,  # marker: H matrices here
    "Win", "Wout", "identF",
    "theta1", "theta2", "b1_col", "b2_col", "w1T", "w2T", "Wl", "bl_rep",
    "ones_row",
]


# --------------------------------------------------------------------------
# device program
# --------------------------------------------------------------------------

def build_program(tc, A, out_ap):
    """Emit the SPMD program. A: dict name -> dram AP. out_ap: [1026,1] f32."""
    import contextlib
    import concourse.bass as bass
    import concourse.mybir as mybir

    nc = tc.nc
    F32 = mybir.dt.float32
    BF16 = mybir.dt.bfloat16
    AF = mybir.ActivationFunctionType
    ALU = mybir.AluOpType
    AX = mybir.AxisListType
    CH3 = ((0, 512), (512, 512), (1024, 128))
    groups = [list(range(NCORES))]

    stack = contextlib.ExitStack()
    CP = stack.enter_context(tc.tile_pool(name="consts", bufs=1))
    WK = stack.enter_context(tc.tile_pool(name="work", bufs=1))
    HP = stack.enter_context(tc.tile_pool(name="hmat", bufs=1))
    DR = stack.enter_context(tc.tile_pool(name="dram", bufs=1, space="DRAM"))

    def load(pool, name, shape, dtype, src_ap):
        t = pool.tile(shape, dtype, name=name)
        nc.sync.dma_start(t[:], src_ap)
        return t

    spec_by_name = dict((s[0], s) for s in _IN_SPECS)
    dtmap = {"f32": F32, "bf16": BF16}
    c = {}
    Hmats = {}
    for nm in _LOAD_ORDER:
        if nm is None:
            for hn in ("Hn_L", "HTe_L", "Hn_G", "HTe_G"):
                tiles = []
                for k in range(NCH):
                    tiles.append(load(HP, f"{hn}_{k}", [128, NP], BF16,
                                      A[hn][k * 128:(k + 1) * 128, :]))
                Hmats[hn] = tiles
            continue
        spec = spec_by_name[nm]
        c[nm] = load(CP, f"c_{nm}", list(spec[1]), dtmap[spec[2]], A[nm][:])

    aeA = load(CP, "aeA", [128, 1], F32, A["ae_col"][0:128])
    aeB = load(CP, "aeB", [16, 1], F32, A["ae_col"][128:SL])
    btA = load(CP, "btA", [128, T], F32, A["bt_sl"][0:128, :])
    btB = load(CP, "btB", [16, T], F32, A["bt_sl"][128:SL, :])

    # broadcast rows -> [64, NP] tiles (partition-broadcast via DMA, last)
    bcast = {}
    for nm, dt_ in (("Binv_L", F32), ("Binv_G", F32),
                    ("Dinv_L", F32), ("Dinv_G", F32)):
        t = CP.tile([64, NP], dt_, name=f"bc_{nm}")
        nc.sync.dma_start(t[:], A[nm][0:1, :].broadcast_to([64, NP]))
        bcast[nm] = t

    identF64 = c["identF"][0:64, 0:64]
    identB64 = c["identB"][0:64, 0:64]

    # ---- persistent work tiles ----
    ctxT = WK.tile([64, T * SL], BF16, name="ctxT")         # [h, (t n)]
    ctx_nA = WK.tile([128, T, 64], BF16, name="ctx_nA")
    ctx_nB = WK.tile([16, T, 64], BF16, name="ctx_nB")
    outT_full = WK.tile([64, NP], BF16, name="outT_full")   # gathered attention out
    x1T = WK.tile([64, NP], BF16, name="x1T")               # L1 out (Dinv deferred)
    x1gT = WK.tile([64, NP], BF16, name="x1gT")             # G1 out (Dinv deferred)
    pay = WK.tile([65, NP], F32, name="pay")                # x2 + S row
    combT2 = WK.tile([128, NP], F32, name="combT2")         # [xgT ; xx1T]

    # ======================= GRU =======================
    with tc.tile_pool(name="sb_gi", bufs=1) as SBGI:
        gi_n = SBGI.tile([64, T * SL], F32, name="gi_n")
        with tc.tile_pool(name="ps_gi", bufs=1, space="PSUM") as PSGI:
            gi_n_ps = PSGI.tile([64, T * SL], F32, name="gi_n_ps", tag="gi")
            for o, w in CH3:
                nc.tensor.matmul(gi_n_ps[:, o:o + w], c["WihT_n"][:],
                                 c["x5"][:, o:o + w], start=True, stop=True)
            nc.scalar.activation(gi_n[:], gi_n_ps[:], AF.Identity,
                                 bias=c["bih_n"][:])

        with tc.tile_pool(name="ps_rz", bufs=3, space="PSUM") as PSR, \
             tc.tile_pool(name="ps_gru", bufs=1, space="PSUM") as PSG, \
             tc.tile_pool(name="sb_gru", bufs=2) as SBG:
            for t in range(T):
                s = slice(t * SL, (t + 1) * SL)
                sp = slice((t - 1) * SL, t * SL)
                rz = SBG.tile([128, SL], F32, name="rz", tag="rz")
                z0 = SBG.tile([64, SL], F32, name="z0", tag="z0")
                wn = SBG.tile([64, SL], F32, name="wn", tag="wn")
                un = SBG.tile([64, SL], F32, name="un", tag="un")
                nt = SBG.tile([64, SL], F32, name="nt", tag="nt")
                mt = SBG.tile([64, SL], F32, name="mt", tag="mt")
                # gates rz accumulated in PSUM: Wih part first (independent
                # of the recurrence), then the Whh part joins the group
                g_rz = PSR.tile([128, SL], F32, name="g_rz", tag="psrz")
                nc.tensor.matmul(g_rz[:], c["WihT_rz"][:], c["x5"][:, s],
                                 start=True, stop=(t == 0))
                if t == 0:
                    nc.scalar.activation(rz[:], g_rz[:], AF.Sigmoid,
                                         bias=c["b_rzsum"][:])
                    nc.scalar.activation(z0[:], rz[64:128, :], AF.Copy)
                    nc.vector.tensor_scalar(wn[:], rz[0:64, :], c["bhh_n"][:],
                                            None, ALU.mult)
                    nc.vector.tensor_tensor(un[:], gi_n[:, s], wn[:], ALU.add)
                    nc.scalar.activation(nt[:], un[:], AF.Tanh)
                    nc.vector.tensor_tensor(mt[:], nt[:], z0[:], ALU.mult)
                    nc.vector.tensor_tensor(ctxT[:, s], nt[:], mt[:],
                                            ALU.subtract)
                else:
                    nc.tensor.matmul(g_rz[:], c["WhhT_rz"][:],
                                     ctxT[:, sp], start=False, stop=True)
                    gh_n = PSG.tile([64, SL], F32, name="gh_n", tag="gh_n")
                    nc.tensor.matmul(gh_n[:], c["WhhT_n"][:], ctxT[:, sp],
                                     start=True, stop=True)
                    nc.scalar.activation(rz[:], g_rz[:], AF.Sigmoid,
                                         bias=c["b_rzsum"][:])
                    nc.scalar.activation(z0[:], rz[64:128, :], AF.Copy)
                    nc.vector.scalar_tensor_tensor(wn[:], gh_n[:],
                                                   c["bhh_n"][:],
                                                   rz[0:64, :], ALU.add,
                                                   ALU.mult)
                    nc.vector.tensor_tensor(un[:], gi_n[:, s], wn[:], ALU.add)
                    nc.scalar.activation(nt[:], un[:], AF.Tanh)
                    dt_ = SBG.tile([64, SL], F32, name="dt_", tag="dt_")
                    nc.vector.tensor_tensor(dt_[:], ctxT[:, sp], nt[:],
                                            ALU.subtract)
                    nc.vector.tensor_tensor(mt[:], dt_[:], z0[:], ALU.mult)
                    nc.vector.tensor_tensor(ctxT[:, s], mt[:], nt[:], ALU.add)
                # node-major ctx for attention via PE transposes
                trA = PSG.tile([128, 64], BF16, name="trA", tag="trA")
                nc.tensor.transpose(trA[:], ctxT[:, t * SL:t * SL + 128],
                                    identB64)
                nc.vector.tensor_copy(ctx_nA[:, t, :], trA[:])
                trB = PSG.tile([16, 64], BF16, name="trB", tag="trB")
                nc.tensor.transpose(trB[:], ctxT[:, t * SL + 128:(t + 1) * SL],
                                    identB64)
                nc.vector.tensor_copy(ctx_nB[:, t, :], trB[:])

    # ======================= attention =======================
    with tc.tile_pool(name="ps_att", bufs=1, space="PSUM") as PSA, \
         tc.tile_pool(name="sb_att", bufs=1) as SBA:
        lastT = ctxT[:, 7 * SL:8 * SL]
        qT_ps = PSA.tile([64, SL], F32, name="qT_ps", tag="qT")
        nc.tensor.matmul(qT_ps[:], c["Win"][:], lastT, start=True, stop=True)
        combT = SBA.tile([128, SL], BF16, name="combT")
        nc.scalar.activation(combT[64:128, :], qT_ps[:], AF.Copy)

        for nm, np_, ctx_n, ae_t, bt_sl, csl in (
                ("A", 128, ctx_nA, aeA[:], btA[:], slice(0, 128)),
                ("B", 16, ctx_nB, aeB[:], btB[:], slice(128, SL))):
            q_ps = PSA.tile([np_, 64], F32, name=f"q_ps{nm}", tag=f"q{nm}")
            nc.tensor.matmul(q_ps[:], lastT[:, csl], c["Win"][:],
                             start=True, stop=True)
            q_s = SBA.tile([np_, 64], F32, name=f"q_s{nm}")
            nc.scalar.activation(q_s[:], q_ps[:], AF.Copy)
            prod = SBA.tile([np_, T, 64], F32, name=f"prod{nm}")
            nc.vector.tensor_tensor(
                prod[:], ctx_n[:],
                q_s[:].unsqueeze(1).broadcast_to([np_, T, 64]), ALU.mult)
            sc = SBA.tile([np_, T], F32, name=f"sc{nm}")
            nc.vector.tensor_reduce(sc[:], prod[:], AX.X, ALU.add)
            den = SBA.tile([np_, 1], F32, name=f"den{nm}")
            ex = SBA.tile([np_, T], F32, name=f"ex{nm}")
            nc.scalar.activation(ex[:], sc[:], AF.Exp, accum_out=den[:])
            rcp = SBA.tile([np_, 1], F32, name=f"rcp{nm}")
            nc.vector.reciprocal(rcp[:], den[:])
            wA = SBA.tile([np_, T], F32, name=f"wA{nm}")
            nc.vector.tensor_scalar(wA[:], ex[:], rcp[:], None, ALU.mult)
            P_t = SBA.tile([np_, T, 64], F32, name=f"P_t{nm}")
            nc.vector.tensor_tensor(
                P_t[:], ctx_n[:],
                wA[:].unsqueeze(2).broadcast_to([np_, T, 64]), ALU.mult)
            G_t = SBA.tile([np_, T, 64], F32, name=f"G_t{nm}")
            nc.vector.tensor_tensor(
                G_t[:], P_t[:],
                bt_sl.unsqueeze(2).broadcast_to([np_, T, 64]), ALU.mult)
            t2_t = SBA.tile([np_, T, 64], F32, name=f"t2_t{nm}")
            nc.scalar.activation(t2_t[:], G_t[:], AF.Relu, scale=ae_t)
            sm = SBA.tile([np_, T, 64], F32, name=f"sm{nm}")
            nc.vector.tensor_tensor(sm[:], P_t[:], t2_t[:], ALU.add)
            mixs = SBA.tile([np_, 64], F32, name=f"mixs{nm}")
            nc.vector.tensor_reduce(
                mixs[:], sm[:].rearrange("p t h -> p h t"), AX.X, ALU.add)
            # transpose mixs into combT rows 0:64
            mtr = PSA.tile([64, np_], F32, name=f"mtr{nm}", tag=f"mtr{nm}")
            nc.tensor.transpose(mtr[:], mixs[:], c["identF"][0:np_, 0:np_])
            nc.scalar.activation(combT[0:64, csl], mtr[:], AF.Copy)

        outT_ps = PSA.tile([64, SL], F32, name="outT_ps", tag="outT")
        nc.tensor.matmul(outT_ps[:], c["Wout"][:], combT[:],
                         start=True, stop=True)
        outT_slice = SBA.tile([64, SL], BF16, name="outT_slice")
        nc.scalar.activation(outT_slice[:], outT_ps[:], AF.Tanh)

        # ---- collective 1: allgather attention output (bf16) ----
        cc1_in = DR.tile([64, SL], BF16, name="cc1_in")
        cc1_out = DR.tile([NCORES, 64, SL], BF16, name="cc1_out",
                          addr_space="Shared")
        nc.sync.dma_start(cc1_in[:], outT_slice[:])
        nc.gpsimd.collective_compute(
            "AllGather", ALU.bypass, replica_groups=groups,
            ins=[cc1_in[:].opt()], outs=[cc1_out[:].opt()])
        nc.sync.dma_start(
            outT_full[:].rearrange("p (c n) -> p c n", c=NCORES),
            cc1_out[:].rearrange("c p n -> p c n"))

    # ======================= hypergraph convs =======================
    conv_stack = contextlib.ExitStack()
    PSX = conv_stack.enter_context(tc.tile_pool(name="ps_xp", bufs=1, space="PSUM"))
    PAcc = conv_stack.enter_context(tc.tile_pool(name="ps_acc", bufs=1, space="PSUM"))
    SBC = conv_stack.enter_context(tc.tile_pool(name="sb_conv", bufs=2))

    EVEN = [k for k in range(NCH) if k % 2 == 0]
    ODD = [k for k in range(NCH) if k % 2 == 1]

    def conv_block(xT_in, theta_t, b_col, Hn_ts, HTe_ts, Binv_bc, Dinv_bc,
                   tag, out_mode, out_dst, S_col=None):
        """One HypergraphConv layer, feature-major, col-tiled matmul pairs."""
        xp_ps = PSX.tile([128, NCH * 64], F32, name=f"xp_{tag}", tag="xp")
        for k in range(NCH):
            nc.tensor.matmul(xp_ps[:, k * 64:(k + 1) * 64],
                             xT_in[:, k * 128:(k + 1) * 128], theta_t[:],
                             start=True, stop=True)
        xpbf = SBC.tile([128, NCH, 64], BF16, name=f"xpbf_{tag}", tag="xpbf")
        nc.scalar.activation(
            xpbf[:], xp_ps[:].rearrange("p (k h) -> p k h", k=NCH), AF.Copy)

        # stage 1: e^T(raw) = xp^T @ Hn, col-tiled even/odd chunk pairs
        eb_ps = PAcc.tile([128, NP], F32, name=f"ebT_{tag}", tag="acc")
        for i in range(len(EVEN)):
            for o, w in CH3:
                k = EVEN[i]
                nc.tensor.matmul(eb_ps[0:64, o:o + w], xpbf[:, k, :],
                                 Hn_ts[k][:, o:o + w],
                                 start=(k == EVEN[0]), stop=(k == EVEN[-1]))
                if i < len(ODD):
                    k = ODD[i]
                    nc.tensor.matmul(eb_ps[64:128, o:o + w], xpbf[:, k, :],
                                     Hn_ts[k][:, o:o + w],
                                     start=(k == ODD[0]), stop=(k == ODD[-1]))
        # combine halves + fold B^-1 (column scale)
        e_top = SBC.tile([64, NP], BF16, name=f"etop_{tag}", tag="etop")
        nc.scalar.activation(e_top[:], eb_ps[0:64, :], AF.Copy)
        e_sum = SBC.tile([64, NP], BF16, name=f"esum_{tag}", tag="esum")
        nc.vector.tensor_tensor(e_sum[:], e_top[:], eb_ps[64:128, :], ALU.add)
        ebTbf = SBC.tile([64, NP], BF16, name=f"ebTbf_{tag}", tag="ebTbf")
        nc.vector.tensor_tensor(ebTbf[:], e_sum[:], Binv_bc[:], ALU.mult)

        # transpose e to edge-major chunks
        tr_ps = PSX.tile([128, NCH * 64], BF16, name=f"tr_{tag}", tag="xp")
        for k in range(NCH):
            nc.tensor.transpose(tr_ps[:, k * 64:(k + 1) * 64],
                                ebTbf[:, k * 128:(k + 1) * 128], identB64)
        ebbf = SBC.tile([128, NCH, 64], BF16, name=f"ebbf_{tag}", tag="ebbf")
        nc.scalar.activation(
            ebbf[:], tr_ps[:].rearrange("p (k h) -> p k h", k=NCH), AF.Copy)

        # stage 2: out^T = D^-1 (e^T @ HTe) + b, then leaky
        oT_ps = PAcc.tile([128, NP], F32, name=f"oT_{tag}", tag="acc")
        for i in range(len(EVEN)):
            for o, w in CH3:
                k = EVEN[i]
                nc.tensor.matmul(oT_ps[0:64, o:o + w], ebbf[:, k, :],
                                 HTe_ts[k][:, o:o + w],
                                 start=(k == EVEN[0]), stop=(k == EVEN[-1]))
                if i < len(ODD):
                    k = ODD[i]
                    nc.tensor.matmul(oT_ps[64:128, o:o + w], ebbf[:, k, :],
                                     HTe_ts[k][:, o:o + w],
                                     start=(k == ODD[0]), stop=(k == ODD[-1]))
        o_top = SBC.tile([64, NP], BF16, name=f"otop_{tag}", tag="otop")
        nc.scalar.activation(o_top[:], oT_ps[0:64, :], AF.Copy)
        u = SBC.tile([64, NP], F32, name=f"u_{tag}", tag="u")
        nc.vector.tensor_tensor(u[:], o_top[:], oT_ps[64:128, :], ALU.add)
        m = SBC.tile([64, NP], F32, name=f"m_{tag}", tag="m")
        nc.vector.tensor_tensor(m[:], u[:], Dinv_bc[:], ALU.mult)
        l1 = SBC.tile([64, NP], F32, name=f"l1_{tag}", tag="lk1")
        nc.vector.tensor_scalar(l1[:], m[:], b_col[:], 0.2, ALU.add, ALU.mult)
        l2 = SBC.tile([64, NP], F32, name=f"l2_{tag}", tag="lk2")
        nc.vector.tensor_scalar(l2[:], m[:], b_col[:], None, ALU.add)
        if S_col is not None:
            nc.vector.scalar_tensor_tensor(out_dst, l2[:], 1.0, l1[:],
                                           ALU.mult, ALU.max,
                                           accum_out=S_col)
        else:
            nc.vector.tensor_tensor(out_dst, l2[:], l1[:], ALU.max)

    S_col = SBC.tile([64, 1], F32, name="S_col")

    # local (timestep) convs: layer 1 then layer 2 -> pay
    conv_block(outT_full[:], c["theta1"], c["b1_col"],
               Hmats["Hn_L"], Hmats["HTe_L"], bcast["Binv_L"],
               bcast["Dinv_L"], "L1", "inter", x1T[:])
    conv_block(x1T[:], c["theta2"], c["b2_col"],
               Hmats["Hn_L"], Hmats["HTe_L"], bcast["Binv_L"],
               bcast["Dinv_L"], "L2", "final", pay[0:64, :], S_col=S_col)

    # S scalar into pay row 64
    nc.vector.memset(pay[64:65, :], 0.0)
    S_tr = PSX.tile([1, 64], F32, name="S_tr", tag="str")
    nc.tensor.transpose(S_tr[:], S_col[:], identF64)
    nc.vector.tensor_reduce(pay[64:65, 0:1], S_tr[:], AX.X, ALU.add)

    # ---- collective 2: allgather conv results + sums (f32) ----
    cc2_in = DR.tile([65, NP], F32, name="cc2_in")
    cc2_out = DR.tile([NCORES, 65, NP], F32, name="cc2_out",
                      addr_space="Shared")
    nc.sync.dma_start(cc2_in[:], pay[:])
    nc.gpsimd.collective_compute(
        "AllGather", ALU.bypass, replica_groups=groups,
        ins=[cc2_in[:].opt()], outs=[cc2_out[:].opt()])

    # global conv (overlaps the collective; result into combT2 rows 0:64)
    conv_block(outT_full[:], c["theta1"], c["b1_col"],
               Hmats["Hn_G"], Hmats["HTe_G"], bcast["Binv_G"],
               bcast["Dinv_G"], "G1", "inter", x1gT[:])
    conv_block(x1gT[:], c["theta2"], c["b2_col"],
               Hmats["Hn_G"], Hmats["HTe_G"], bcast["Binv_G"],
               bcast["Dinv_G"], "G2", "final", combT2[0:64, :])

    conv_stack.close()

    # ======================= final stage =======================
    with tc.tile_pool(name="sb_fin", bufs=1) as SBF, \
         tc.tile_pool(name="ps_fin", bufs=1, space="PSUM") as PSF:
        # temporal attention weights from the gathered S values
        Sg0 = SBF.tile([T - 1, 1], F32, name="Sg0")
        nc.sync.dma_start(Sg0[:], cc2_out[0:T - 1, 64, 0:1])
        Sg1 = SBF.tile([T - 1, 1], F32, name="Sg1")
        nc.sync.dma_start(Sg1[:], cc2_out[1:T, 64, 0:1])
        zv = SBF.tile([T - 1, 1], F32, name="zv")
        nc.vector.tensor_tensor(zv[:], Sg1[:], Sg0[:], ALU.subtract)
        y_ps = PSF.tile([64, 1], F32, name="y_ps", tag="str")
        nc.tensor.matmul(y_ps[:], c["w1T"][:], zv[:], start=True, stop=True)
        y1 = SBF.tile([64, 1], F32, name="y1")
        nc.vector.tensor_scalar(y1[:], y_ps[:], 0.2, None, ALU.mult)
        y_s = SBF.tile([64, 1], F32, name="y_s")
        nc.vector.tensor_tensor(y_s[:], y_ps[:], y1[:], ALU.max)
        wat_ps = PSF.tile([T - 1, 1], F32, name="wat_ps", tag="str")
        nc.tensor.matmul(wat_ps[:], c["w2T"][:], y_s[:], start=True, stop=True)
        wat_s = SBF.tile([T - 1, 1], F32, name="wat_s")
        nc.vector.tensor_copy(wat_s[:], wat_ps[:])
        watT_ps = PSF.tile([1, T - 1], F32, name="watT_ps", tag="str")
        nc.tensor.transpose(watT_ps[:], wat_s[:], c["identF"][0:7, 0:7])
        nmw = SBF.tile([1, 1], F32, name="nmw")
        nc.vector.tensor_reduce(nmw[:], watT_ps[:], AX.X, ALU.max, negate=True)
        den = SBF.tile([1, 1], F32, name="den")
        exw = SBF.tile([1, T - 1], F32, name="exw")
        nc.scalar.activation(exw[:], watT_ps[:], AF.Exp, bias=nmw[:],
                             accum_out=den[:])
        rw = SBF.tile([1, 1], F32, name="rw")
        nc.vector.reciprocal(rw[:], den[:])
        wsm = SBF.tile([1, T - 1], F32, name="wsm")
        nc.vector.tensor_scalar(wsm[:], exw[:], rw[:], None, ALU.mult)
        # broadcast wsm across 128 partitions via a K=1 matmul
        wbc_ps = PSF.tile([128, T - 1], F32, name="wbc_ps", tag="str")
        nc.tensor.matmul(wbc_ps[:], c["ones_row"][:], wsm[:],
                         start=True, stop=True)
        wbc = SBF.tile([128, T - 1], F32, name="wbc")
        nc.vector.tensor_copy(wbc[:], wbc_ps[:])

        # xx1 = w0*(x2[1]-x2[0]) + w2*(x2[3]-x2[2])
        x2 = []
        for t_ in range(4):
            xt_ = SBF.tile([64, NP], F32, name=f"x2_{t_}")
            nc.sync.dma_start(xt_[:], cc2_out[t_, 0:64, :])
            x2.append(xt_)
        d0 = SBF.tile([64, NP], F32, name="d0")
        nc.vector.tensor_tensor(d0[:], x2[1][:], x2[0][:], ALU.subtract)
        d2 = SBF.tile([64, NP], F32, name="d2")
        nc.vector.tensor_tensor(d2[:], x2[3][:], x2[2][:], ALU.subtract)
        m0 = SBF.tile([64, NP], F32, name="m0")
        nc.vector.tensor_scalar(m0[:], d0[:], wbc[0:64, 0:1], None, ALU.mult)
        nc.vector.scalar_tensor_tensor(combT2[64:128, :], d2[:],
                                       wbc[0:64, 2:3], m0[:],
                                       ALU.mult, ALU.add)
        # output head: res[n] = leaky(Wl . comb[:, n] + bl)
        res_ps = PSF.tile([128, NCH], F32, name="res_ps", tag="str")
        for k in range(NCH):
            nc.tensor.matmul(res_ps[:, k:k + 1],
                             combT2[:, k * 128:(k + 1) * 128], c["Wl"][:],
                             start=True, stop=True)
        r1 = SBF.tile([128, NCH], F32, name="r1")
        nc.vector.tensor_scalar(r1[:], res_ps[:], c["bl_rep"][:], 0.2,
                                ALU.add, ALU.mult)
        r2 = SBF.tile([128, NCH], F32, name="r2")
        nc.vector.tensor_scalar(r2[:], res_ps[:], c["bl_rep"][:], None,
                                ALU.add)
        res_s = SBF.tile([128, NCH], F32, name="res_s")
        nc.vector.tensor_tensor(res_s[:], r2[:], r1[:], ALU.max)
        nc.sync.dma_start(
            out_ap[0:1024, 0:1].rearrange("(k p) o -> p k o", p=128),
            res_s[:, 0:8].unsqueeze(2))
        nc.sync.dma_start(out_ap[1024:1026, 0:1], res_s[0:2, 8:9])

    stack.close()


# --------------------------------------------------------------------------
# entry points
# --------------------------------------------------------------------------

def _make_nc():
    if "nc" in _NC_CACHE:
        return _NC_CACHE["nc"]
    import concourse.bacc as bacc
    import concourse.mybir as mybir
    from concourse import tile

    nc = bacc.Bacc("TRN2", target_bir_lowering=False, debug=False,
                   enable_asserts=True, num_devices=NCORES)
    A = {}
    dtmap = {"f32": mybir.dt.float32, "bf16": mybir.dt.bfloat16}
    for nm, shape, dt_ in _IN_SPECS:
        A[nm] = nc.dram_tensor(
            nm, list(shape), dtmap[dt_], kind="ExternalInput").ap()
    out_h = nc.dram_tensor("out", [N, 1], mybir.dt.float32,
                           kind="ExternalOutput")
    with tile.TileContext(nc) as tc:
        build_program(tc, A, out_h.ap())
    nc.compile()
    _NC_CACHE["nc"] = nc
    return nc


def kernel(**inputs):
    from concourse.bass_utils import run_bass_kernel_spmd
    nc = _make_nc()
    in_maps = _host_prep(inputs)
    res = run_bass_kernel_spmd(nc, in_maps, list(range(NCORES)))
    return np.asarray(res.results[0]["out"])


# revision 25
# speedup vs baseline: 1.2682x; 1.0236x over previous
"""HGAT (GRU + decayed attention + 2x HypergraphConv over 9 hypergraphs) on 8 trn2 cores.

Strategy:
  - Host: densify each hypergraph incidence list into dense [1152,1152]
    operators holding RAW integer counts (exact in bf16), shipped in both
    layouts (node-major / edge-major).  B^-1 / D^-1 / bias scalings are folded
    into per-column vector ops / rank-1 matmuls on device.  The attention
    decay factors exp(-ab*delta) are precomputed on host.
  - Device (SPMD, 8 cores): GRU+attention sharded over nodes (144/core) with
    gates accumulated directly in PSUM (bf16 matmuls), AllGather (bf16) the
    attention output, data-parallel hypergraph convs (core c: timestep c)
    with column-tiled matmul pairs (2 concurrent 64-wide output groups),
    AllGather timestep conv results + per-timestep sums (f32), the
    global-hyp conv computed redundantly on all cores DURING the second
    AllGather, final temporal attention + output head with a PE-broadcast of
    softmax weights (no DRAM bounce).
"""
import numpy as np
import ml_dtypes

N, T, H, F_IN, E = 1026, 8, 64, 5, 1026
NP = 1152            # padded N and E (9 * 128)
NCORES = 8
SL = NP // NCORES    # 144 nodes per core
NCH = NP // 128      # 9 contraction chunks
BF = ml_dtypes.bfloat16

_NC_CACHE = {}


# --------------------------------------------------------------------------
# host-side prep
# --------------------------------------------------------------------------

def _densify(idx):
    node = idx[0].astype(np.int64)
    edge = idx[1].astype(np.int64)
    Hm = np.bincount(node * NP + edge, minlength=N * NP).reshape(N, NP)
    Hp = np.zeros((NP, NP), np.float32)
    Hp[:N] = Hm.astype(np.float32)
    degn = Hp.sum(1)
    dege = Hp.sum(0)
    Dinv = np.where(degn > 0, 1.0 / degn, 0.0).astype(np.float32)
    Binv = np.where(dege > 0, 1.0 / dege, 0.0).astype(np.float32)
    Hn = np.ascontiguousarray(Hp.astype(BF))                  # [n, e] raw counts
    HTe = np.ascontiguousarray(Hp.T.astype(BF))               # [e, n] raw counts
    return Hn, HTe, degn, Dinv, Binv


def _host_prep(inp):
    f32 = np.float32
    price = np.asarray(inp["price_input"], f32)          # [N, T, F]
    hyp_T = np.asarray(inp["hyp_T"])                     # [T, 2, nnz]
    hyp = np.asarray(inp["hyp"])                         # [2, nnz]

    WihT = np.ascontiguousarray(np.asarray(inp["Wih"], f32).T)   # [5, 192]
    WhhT = np.ascontiguousarray(np.asarray(inp["Whh"], f32).T)   # [64, 192]
    bih = np.asarray(inp["bih"], f32)
    bhh = np.asarray(inp["bhh"], f32)

    shared = {
        "WihT_rz": np.ascontiguousarray(WihT[:, 0:128]).astype(BF),
        "WihT_n": np.ascontiguousarray(WihT[:, 128:192]).astype(BF),
        "WhhT_rz": np.ascontiguousarray(WhhT[:, 0:128]).astype(BF),
        "WhhT_n": np.ascontiguousarray(WhhT[:, 128:192]).astype(BF),
        "b_rzsum": np.ascontiguousarray((bih[0:128] + bhh[0:128])[:, None]),
        "bih_n": np.ascontiguousarray(bih[128:192, None]),
        "bhh_n": np.ascontiguousarray(bhh[128:192, None]),
        "Win": np.asarray(inp["Win"], BF),
        "Wout": np.asarray(inp["Wout"], BF),
        "theta1": np.asarray(inp["theta1"], BF),
        "theta2": np.asarray(inp["theta2"], BF),
        "b1_col": np.ascontiguousarray(np.asarray(inp["bias1"], f32)[:, None]),
        "b2_col": np.ascontiguousarray(np.asarray(inp["bias2"], f32)[:, None]),
        "w1T": np.ascontiguousarray(np.asarray(inp["w1"], f32).T),   # [7, 64]
        "w2T": np.ascontiguousarray(np.asarray(inp["w2"], f32).T),   # [64, 7]
        "Wl": np.asarray(inp["Wl"], BF),                            # [128, 1]
        "bl_rep": np.full((128, 1), np.asarray(inp["bl"], f32)[0], f32),
        "ones_row": np.ones((1, 128), f32),
        "identF": np.eye(128, dtype=f32),
        "identB": np.eye(128, dtype=BF),
    }

    HnG, HTeG, degG, DinvG, BinvG = _densify(hyp)
    shared["Hn_G"] = HnG
    shared["HTe_G"] = HTeG
    shared["Binv_G"] = np.ascontiguousarray(BinvG[None, :])
    shared["Dinv_G"] = np.ascontiguousarray(DinvG[None, :])

    price_p = np.zeros((NP, T, F_IN), f32)
    price_p[:N] = price
    ae_p = np.zeros((NP,), f32)
    ae_p[:N] = np.asarray(inp["ae"], f32)[:, 0, 0]
    ab_p = np.zeros((NP,), f32)
    ab_p[:N] = np.asarray(inp["ab"], f32)[:, 0, 0]
    delta = np.arange(T - 1, -1, -1, dtype=f32)
    bt_full = np.exp(-ab_p[:, None] * delta[None, :])    # [NP, T]

    in_maps = []
    for c in range(NCORES):
        sl = slice(c * SL, (c + 1) * SL)
        m = dict(shared)
        m["x5"] = np.ascontiguousarray(
            price_p[sl].transpose(2, 1, 0).reshape(F_IN, T * SL)).astype(BF)
        m["ae_col"] = np.ascontiguousarray(ae_p[sl, None])
        m["bt_sl"] = np.ascontiguousarray(bt_full[sl])
        HnL, HTeL, degL, DinvL, BinvL = _densify(hyp_T[c])
        m["Hn_L"] = HnL
        m["HTe_L"] = HTeL
        m["Binv_L"] = np.ascontiguousarray(BinvL[None, :])
        m["Dinv_L"] = np.ascontiguousarray(DinvL[None, :])
        in_maps.append(m)
    return in_maps


_IN_SPECS = [
    ("x5", (F_IN, NP), "bf16"),
    ("WihT_rz", (F_IN, 128), "bf16"), ("WihT_n", (F_IN, 64), "bf16"),
    ("WhhT_rz", (64, 128), "bf16"), ("WhhT_n", (64, 64), "bf16"),
    ("b_rzsum", (128, 1), "f32"),
    ("bih_n", (64, 1), "f32"), ("bhh_n", (64, 1), "f32"),
    ("identB", (128, 128), "bf16"),
    ("Hn_L", (NP, NP), "bf16"), ("HTe_L", (NP, NP), "bf16"),
    ("Hn_G", (NP, NP), "bf16"), ("HTe_G", (NP, NP), "bf16"),
    ("Win", (64, 64), "bf16"), ("Wout", (128, 64), "bf16"),
    ("ae_col", (SL, 1), "f32"), ("bt_sl", (SL, T), "f32"),
    ("identF", (128, 128), "f32"),
    ("theta1", (64, 64), "bf16"), ("theta2", (64, 64), "bf16"),
    ("b1_col", (64, 1), "f32"), ("b2_col", (64, 1), "f32"),
    ("w1T", (T - 1, 64), "f32"), ("w2T", (64, T - 1), "f32"),
    ("Wl", (128, 1), "bf16"), ("bl_rep", (128, 1), "f32"),
    ("ones_row", (1, 128), "f32"),
    ("Binv_L", (1, NP), "f32"), ("Binv_G", (1, NP), "f32"),
    ("Dinv_L", (1, NP), "f32"), ("Dinv_G", (1, NP), "f32"),
]

# DMA issue order: GRU-critical first, then H operators, then attention
# consts, then conv/final consts (single in-order DMA queue).
_LOAD_ORDER = [
    "x5", "WihT_rz", "WihT_n", "WhhT_rz", "WhhT_n", "b_rzsum", "bih_n",
    "bhh_n", "identB",
    None,  # marker: H matrices here
    "Win", "Wout", "identF",
    "theta1", "theta2", "b1_col", "b2_col", "w1T", "w2T", "Wl", "bl_rep",
    "ones_row",
]


# --------------------------------------------------------------------------
# device program
# --------------------------------------------------------------------------

def build_program(tc, A, out_ap):
    """Emit the SPMD program. A: dict name -> dram AP. out_ap: [1026,1] f32."""
    import contextlib
    import concourse.bass as bass
    import concourse.mybir as mybir

    nc = tc.nc
    F32 = mybir.dt.float32
    BF16 = mybir.dt.bfloat16
    AF = mybir.ActivationFunctionType
    ALU = mybir.AluOpType
    AX = mybir.AxisListType
    CH3 = ((0, 512), (512, 512), (1024, 128))
    groups = [list(range(NCORES))]

    stack = contextlib.ExitStack()
    CP = stack.enter_context(tc.tile_pool(name="consts", bufs=1))
    WK = stack.enter_context(tc.tile_pool(name="work", bufs=1))
    HP = stack.enter_context(tc.tile_pool(name="hmat", bufs=1))
    DR = stack.enter_context(tc.tile_pool(name="dram", bufs=1, space="DRAM"))

    def load(pool, name, shape, dtype, src_ap):
        t = pool.tile(shape, dtype, name=name)
        nc.sync.dma_start(t[:], src_ap)
        return t

    spec_by_name = dict((s[0], s) for s in _IN_SPECS)
    dtmap = {"f32": F32, "bf16": BF16}
    c = {}
    Hmats = {}
    for nm in _LOAD_ORDER:
        if nm is None:
            for hn in ("Hn_L", "HTe_L", "Hn_G", "HTe_G"):
                tiles = []
                for k in range(NCH):
                    tiles.append(load(HP, f"{hn}_{k}", [128, NP], BF16,
                                      A[hn][k * 128:(k + 1) * 128, :]))
                Hmats[hn] = tiles
            continue
        spec = spec_by_name[nm]
        c[nm] = load(CP, f"c_{nm}", list(spec[1]), dtmap[spec[2]], A[nm][:])

    aeA = load(CP, "aeA", [128, 1], F32, A["ae_col"][0:128])
    aeB = load(CP, "aeB", [16, 1], F32, A["ae_col"][128:SL])
    btA = load(CP, "btA", [128, T], F32, A["bt_sl"][0:128, :])
    btB = load(CP, "btB", [16, T], F32, A["bt_sl"][128:SL, :])

    # broadcast rows -> [64, NP] tiles (partition-broadcast via DMA, last)
    bcast = {}
    for nm, dt_ in (("Binv_L", F32), ("Binv_G", F32),
                    ("Dinv_L", F32), ("Dinv_G", F32)):
        t = CP.tile([64, NP], dt_, name=f"bc_{nm}")
        nc.sync.dma_start(t[:], A[nm][0:1, :].broadcast_to([64, NP]))
        bcast[nm] = t

    identF64 = c["identF"][0:64, 0:64]
    identB64 = c["identB"][0:64, 0:64]

    # ---- persistent work tiles ----
    ctxT = WK.tile([64, T * SL], BF16, name="ctxT")         # [h, (t n)]
    ctx_nA = WK.tile([128, T, 64], BF16, name="ctx_nA")
    ctx_nB = WK.tile([16, T, 64], BF16, name="ctx_nB")
    outT_full = WK.tile([64, NP], BF16, name="outT_full")   # gathered attention out
    x1T = WK.tile([64, NP], BF16, name="x1T")               # L1 out (Dinv deferred)
    x1gT = WK.tile([64, NP], BF16, name="x1gT")             # G1 out (Dinv deferred)
    pay = WK.tile([65, NP], F32, name="pay")                # x2 + S row
    combT2 = WK.tile([128, NP], BF16, name="combT2")         # [xgT ; xx1T]

    # ======================= GRU =======================
    with tc.tile_pool(name="sb_gi", bufs=1) as SBGI:
        gi_n = SBGI.tile([64, T * SL], F32, name="gi_n")
        with tc.tile_pool(name="ps_gi", bufs=1, space="PSUM") as PSGI:
            gi_n_ps = PSGI.tile([64, T * SL], F32, name="gi_n_ps", tag="gi")
            for o, w in CH3:
                nc.tensor.matmul(gi_n_ps[:, o:o + w], c["WihT_n"][:],
                                 c["x5"][:, o:o + w], start=True, stop=True)
            nc.scalar.activation(gi_n[:], gi_n_ps[:], AF.Identity,
                                 bias=c["bih_n"][:])

        with tc.tile_pool(name="ps_rz", bufs=3, space="PSUM") as PSR, \
             tc.tile_pool(name="ps_gru", bufs=1, space="PSUM") as PSG, \
             tc.tile_pool(name="sb_gru", bufs=2) as SBG:
            for t in range(T):
                s = slice(t * SL, (t + 1) * SL)
                sp = slice((t - 1) * SL, t * SL)
                rz = SBG.tile([128, SL], F32, name="rz", tag="rz")
                z0 = SBG.tile([64, SL], F32, name="z0", tag="z0")
                wn = SBG.tile([64, SL], F32, name="wn", tag="wn")
                un = SBG.tile([64, SL], F32, name="un", tag="un")
                nt = SBG.tile([64, SL], F32, name="nt", tag="nt")
                mt = SBG.tile([64, SL], F32, name="mt", tag="mt")
                # gates rz accumulated in PSUM: Wih part first (independent
                # of the recurrence), then the Whh part joins the group
                g_rz = PSR.tile([128, SL], F32, name="g_rz", tag="psrz")
                nc.tensor.matmul(g_rz[:], c["WihT_rz"][:], c["x5"][:, s],
                                 start=True, stop=(t == 0))
                if t == 0:
                    nc.scalar.activation(rz[:], g_rz[:], AF.Sigmoid,
                                         bias=c["b_rzsum"][:])
                    nc.scalar.activation(z0[:], rz[64:128, :], AF.Copy)
                    nc.vector.tensor_scalar(wn[:], rz[0:64, :], c["bhh_n"][:],
                                            None, ALU.mult)
                    nc.vector.tensor_tensor(un[:], gi_n[:, s], wn[:], ALU.add)
                    nc.scalar.activation(nt[:], un[:], AF.Tanh)
                    nc.vector.tensor_tensor(mt[:], nt[:], z0[:], ALU.mult)
                    nc.vector.tensor_tensor(ctxT[:, s], nt[:], mt[:],
                                            ALU.subtract)
                else:
                    nc.tensor.matmul(g_rz[:], c["WhhT_rz"][:],
                                     ctxT[:, sp], start=False, stop=True)
                    gh_n = PSG.tile([64, SL], F32, name="gh_n", tag="gh_n")
                    nc.tensor.matmul(gh_n[:], c["WhhT_n"][:], ctxT[:, sp],
                                     start=True, stop=True)
                    nc.scalar.activation(rz[:], g_rz[:], AF.Sigmoid,
                                         bias=c["b_rzsum"][:])
                    nc.scalar.activation(z0[:], rz[64:128, :], AF.Copy)
                    nc.vector.scalar_tensor_tensor(wn[:], gh_n[:],
                                                   c["bhh_n"][:],
                                                   rz[0:64, :], ALU.add,
                                                   ALU.mult)
                    nc.vector.tensor_tensor(un[:], gi_n[:, s], wn[:], ALU.add)
                    nc.scalar.activation(nt[:], un[:], AF.Tanh)
                    dt_ = SBG.tile([64, SL], F32, name="dt_", tag="dt_")
                    nc.vector.tensor_tensor(dt_[:], ctxT[:, sp], nt[:],
                                            ALU.subtract)
                    nc.vector.tensor_tensor(mt[:], dt_[:], z0[:], ALU.mult)
                    nc.vector.tensor_tensor(ctxT[:, s], mt[:], nt[:], ALU.add)
                # node-major ctx for attention via PE transposes
                trA = PSG.tile([128, 64], BF16, name="trA", tag="trA")
                nc.tensor.transpose(trA[:], ctxT[:, t * SL:t * SL + 128],
                                    identB64)
                nc.vector.tensor_copy(ctx_nA[:, t, :], trA[:])
                trB = PSG.tile([16, 64], BF16, name="trB", tag="trB")
                nc.tensor.transpose(trB[:], ctxT[:, t * SL + 128:(t + 1) * SL],
                                    identB64)
                nc.vector.tensor_copy(ctx_nB[:, t, :], trB[:])

    # ======================= attention =======================
    with tc.tile_pool(name="ps_att", bufs=1, space="PSUM") as PSA, \
         tc.tile_pool(name="sb_att", bufs=1) as SBA:
        lastT = ctxT[:, 7 * SL:8 * SL]
        qT_ps = PSA.tile([64, SL], F32, name="qT_ps", tag="qT")
        nc.tensor.matmul(qT_ps[:], c["Win"][:], lastT, start=True, stop=True)
        combT = SBA.tile([128, SL], BF16, name="combT")
        nc.scalar.activation(combT[64:128, :], qT_ps[:], AF.Copy)

        for nm, np_, ctx_n, ae_t, bt_sl, csl in (
                ("A", 128, ctx_nA, aeA[:], btA[:], slice(0, 128)),
                ("B", 16, ctx_nB, aeB[:], btB[:], slice(128, SL))):
            q_ps = PSA.tile([np_, 64], F32, name=f"q_ps{nm}", tag=f"q{nm}")
            nc.tensor.matmul(q_ps[:], lastT[:, csl], c["Win"][:],
                             start=True, stop=True)
            q_s = SBA.tile([np_, 64], F32, name=f"q_s{nm}")
            nc.scalar.activation(q_s[:], q_ps[:], AF.Copy)
            prod = SBA.tile([np_, T, 64], F32, name=f"prod{nm}")
            nc.vector.tensor_tensor(
                prod[:], ctx_n[:],
                q_s[:].unsqueeze(1).broadcast_to([np_, T, 64]), ALU.mult)
            sc = SBA.tile([np_, T], F32, name=f"sc{nm}")
            nc.vector.tensor_reduce(sc[:], prod[:], AX.X, ALU.add)
            den = SBA.tile([np_, 1], F32, name=f"den{nm}")
            ex = SBA.tile([np_, T], F32, name=f"ex{nm}")
            nc.scalar.activation(ex[:], sc[:], AF.Exp, accum_out=den[:])
            rcp = SBA.tile([np_, 1], F32, name=f"rcp{nm}")
            nc.vector.reciprocal(rcp[:], den[:])
            wA = SBA.tile([np_, T], F32, name=f"wA{nm}")
            nc.vector.tensor_scalar(wA[:], ex[:], rcp[:], None, ALU.mult)
            P_t = SBA.tile([np_, T, 64], F32, name=f"P_t{nm}")
            nc.vector.tensor_tensor(
                P_t[:], ctx_n[:],
                wA[:].unsqueeze(2).broadcast_to([np_, T, 64]), ALU.mult)
            G_t = SBA.tile([np_, T, 64], F32, name=f"G_t{nm}")
            nc.vector.tensor_tensor(
                G_t[:], P_t[:],
                bt_sl.unsqueeze(2).broadcast_to([np_, T, 64]), ALU.mult)
            t2_t = SBA.tile([np_, T, 64], F32, name=f"t2_t{nm}")
            nc.scalar.activation(t2_t[:], G_t[:], AF.Relu, scale=ae_t)
            sm = SBA.tile([np_, T, 64], F32, name=f"sm{nm}")
            nc.vector.tensor_tensor(sm[:], P_t[:], t2_t[:], ALU.add)
            mixs = SBA.tile([np_, 64], F32, name=f"mixs{nm}")
            nc.vector.tensor_reduce(
                mixs[:], sm[:].rearrange("p t h -> p h t"), AX.X, ALU.add)
            # transpose mixs into combT rows 0:64
            mtr = PSA.tile([64, np_], F32, name=f"mtr{nm}", tag=f"mtr{nm}")
            nc.tensor.transpose(mtr[:], mixs[:], c["identF"][0:np_, 0:np_])
            nc.scalar.activation(combT[0:64, csl], mtr[:], AF.Copy)

        outT_ps = PSA.tile([64, SL], F32, name="outT_ps", tag="outT")
        nc.tensor.matmul(outT_ps[:], c["Wout"][:], combT[:],
                         start=True, stop=True)
        outT_slice = SBA.tile([64, SL], BF16, name="outT_slice")
        nc.scalar.activation(outT_slice[:], outT_ps[:], AF.Tanh)

        # ---- collective 1: allgather attention output (bf16) ----
        cc1_in = DR.tile([64, SL], BF16, name="cc1_in")
        cc1_out = DR.tile([NCORES, 64, SL], BF16, name="cc1_out",
                          addr_space="Shared")
        nc.sync.dma_start(cc1_in[:], outT_slice[:])
        nc.gpsimd.collective_compute(
            "AllGather", ALU.bypass, replica_groups=groups,
            ins=[cc1_in[:].opt()], outs=[cc1_out[:].opt()])
        nc.sync.dma_start(
            outT_full[:].rearrange("p (c n) -> p c n", c=NCORES),
            cc1_out[:].rearrange("c p n -> p c n"))

    # ======================= hypergraph convs =======================
    conv_stack = contextlib.ExitStack()
    PSX = conv_stack.enter_context(tc.tile_pool(name="ps_xp", bufs=1, space="PSUM"))
    PAcc = conv_stack.enter_context(tc.tile_pool(name="ps_acc", bufs=1, space="PSUM"))
    SBC = conv_stack.enter_context(tc.tile_pool(name="sb_conv", bufs=1))

    EVEN = [k for k in range(NCH) if k % 2 == 0]
    ODD = [k for k in range(NCH) if k % 2 == 1]

    def cb_front(xT_in, theta_t, Hn_ts, Binv_bc, tag, acc):
        """xp = theta^T x; e^T = B^-1 (xp^T Hn); edge-major bf16 chunks."""
        xp_ps = PSX.tile([128, NCH * 64], F32, name=f"xp_{tag}", tag="xp")
        for k in range(NCH):
            nc.tensor.matmul(xp_ps[:, k * 64:(k + 1) * 64],
                             xT_in[:, k * 128:(k + 1) * 128], theta_t[:],
                             start=True, stop=True)
        xpbf = SBC.tile([128, NCH, 64], BF16, name=f"xpbf_{tag}", tag="xpbf")
        nc.scalar.activation(
            xpbf[:], xp_ps[:].rearrange("p (k h) -> p k h", k=NCH), AF.Copy)
        eb_ps = PAcc.tile([128, NP], F32, name=f"ebT_{tag}", tag=acc)
        for i in range(len(EVEN)):
            for o, w in CH3:
                k = EVEN[i]
                nc.tensor.matmul(eb_ps[0:64, o:o + w], xpbf[:, k, :],
                                 Hn_ts[k][:, o:o + w],
                                 start=(k == EVEN[0]), stop=(k == EVEN[-1]))
                if i < len(ODD):
                    k = ODD[i]
                    nc.tensor.matmul(eb_ps[64:128, o:o + w], xpbf[:, k, :],
                                     Hn_ts[k][:, o:o + w],
                                     start=(k == ODD[0]), stop=(k == ODD[-1]))
        e_top = SBC.tile([64, NP], BF16, name=f"etop_{tag}", tag="etop")
        nc.scalar.activation(e_top[:], eb_ps[0:64, :], AF.Copy)
        e_sum = SBC.tile([64, NP], F32, name=f"esum_{tag}", tag="esum")
        nc.vector.tensor_tensor(e_sum[:], e_top[:], eb_ps[64:128, :], ALU.add)
        ebTbf = SBC.tile([64, NP], BF16, name=f"ebTbf_{tag}", tag="ebTbf")
        nc.vector.tensor_tensor(ebTbf[:], e_sum[:], Binv_bc[:], ALU.mult)
        tr_ps = PSX.tile([128, NCH * 64], BF16, name=f"tr_{tag}", tag="xp")
        for k in range(NCH):
            nc.tensor.transpose(tr_ps[:, k * 64:(k + 1) * 64],
                                ebTbf[:, k * 128:(k + 1) * 128], identB64)
        ebbf = SBC.tile([128, NCH, 64], BF16, name=f"ebbf_{tag}", tag=f"eb{tag}")
        nc.scalar.activation(
            ebbf[:], tr_ps[:].rearrange("p (k h) -> p k h", k=NCH), AF.Copy)
        return ebbf

    def cb_back(ebbf, HTe_ts, b_col, Dinv_bc, tag, acc, out_dst, S_col=None):
        """out^T = leaky(D^-1 (e^T HTe) + b)."""
        oT_ps = PAcc.tile([128, NP], F32, name=f"oT_{tag}", tag=acc)
        for i in range(len(EVEN)):
            for o, w in CH3:
                k = EVEN[i]
                nc.tensor.matmul(oT_ps[0:64, o:o + w], ebbf[:, k, :],
                                 HTe_ts[k][:, o:o + w],
                                 start=(k == EVEN[0]), stop=(k == EVEN[-1]))
                if i < len(ODD):
                    k = ODD[i]
                    nc.tensor.matmul(oT_ps[64:128, o:o + w], ebbf[:, k, :],
                                     HTe_ts[k][:, o:o + w],
                                     start=(k == ODD[0]), stop=(k == ODD[-1]))
        o_top = SBC.tile([64, NP], BF16, name=f"otop_{tag}", tag="otop")
        nc.scalar.activation(o_top[:], oT_ps[0:64, :], AF.Copy)
        u = SBC.tile([64, NP], F32, name=f"u_{tag}", tag="u")
        nc.vector.tensor_tensor(u[:], o_top[:], oT_ps[64:128, :], ALU.add)
        m = SBC.tile([64, NP], F32, name=f"m_{tag}", tag="m")
        nc.vector.tensor_tensor(m[:], u[:], Dinv_bc[:], ALU.mult)
        l1 = SBC.tile([64, NP], F32, name=f"l1_{tag}", tag="lk1")
        nc.vector.tensor_scalar(l1[:], m[:], b_col[:], 0.2, ALU.add, ALU.mult)
        l2 = SBC.tile([64, NP], F32, name=f"l2_{tag}", tag="lk2")
        nc.vector.tensor_scalar(l2[:], m[:], b_col[:], None, ALU.add)
        if S_col is not None:
            nc.vector.scalar_tensor_tensor(out_dst, l2[:], 1.0, l1[:],
                                           ALU.mult, ALU.max,
                                           accum_out=S_col)
        else:
            nc.vector.tensor_tensor(out_dst, l2[:], l1[:], ALU.max)

    S_col = SBC.tile([64, 1], F32, name="S_col")

    # interleaved L/G conv emission: G fills PE bubbles of L epilogues;
    # G2's back half runs under the second AllGather.
    ebbfL1 = cb_front(outT_full[:], c["theta1"], Hmats["Hn_L"],
                      bcast["Binv_L"], "L1", "accL")
    ebbfG1 = cb_front(outT_full[:], c["theta1"], Hmats["Hn_G"],
                      bcast["Binv_G"], "G1", "accG")
    cb_back(ebbfL1, Hmats["HTe_L"], c["b1_col"], bcast["Dinv_L"], "L1",
            "accL", x1T[:])
    cb_back(ebbfG1, Hmats["HTe_G"], c["b1_col"], bcast["Dinv_G"], "G1",
            "accG", x1gT[:])
    ebbfL2 = cb_front(x1T[:], c["theta2"], Hmats["Hn_L"],
                      bcast["Binv_L"], "L2", "accL")
    ebbfG2 = cb_front(x1gT[:], c["theta2"], Hmats["Hn_G"],
                      bcast["Binv_G"], "G2", "accG")
    cb_back(ebbfL2, Hmats["HTe_L"], c["b2_col"], bcast["Dinv_L"], "L2",
            "accL", pay[0:64, :], S_col=S_col)

    # S scalar into pay row 64
    nc.vector.memset(pay[64:65, :], 0.0)
    S_tr = PSX.tile([1, 64], F32, name="S_tr", tag="xp")
    nc.tensor.transpose(S_tr[:], S_col[:], identF64)
    nc.vector.tensor_reduce(pay[64:65, 0:1], S_tr[:], AX.X, ALU.add)

    # ---- collective 2: allgather conv results + sums (f32) ----
    cc2_in = DR.tile([65, NP], F32, name="cc2_in")
    cc2_out = DR.tile([NCORES, 65, NP], F32, name="cc2_out",
                      addr_space="Shared")
    nc.sync.dma_start(cc2_in[:], pay[:])
    nc.gpsimd.collective_compute(
        "AllGather", ALU.bypass, replica_groups=groups,
        ins=[cc2_in[:].opt()], outs=[cc2_out[:].opt()])

    # global conv layer-2 back half overlaps the collective
    cb_back(ebbfG2, Hmats["HTe_G"], c["b2_col"], bcast["Dinv_G"], "G2",
            "accG", combT2[0:64, :])

    conv_stack.close()

    # ======================= final stage =======================
    with tc.tile_pool(name="sb_fin", bufs=1) as SBF, \
         tc.tile_pool(name="ps_fin", bufs=1, space="PSUM") as PSF:
        # temporal attention weights from the gathered S values
        Sg0 = SBF.tile([T - 1, 1], F32, name="Sg0")
        nc.sync.dma_start(Sg0[:], cc2_out[0:T - 1, 64, 0:1])
        Sg1 = SBF.tile([T - 1, 1], F32, name="Sg1")
        nc.sync.dma_start(Sg1[:], cc2_out[1:T, 64, 0:1])
        zv = SBF.tile([T - 1, 1], F32, name="zv")
        nc.vector.tensor_tensor(zv[:], Sg1[:], Sg0[:], ALU.subtract)
        y_ps = PSF.tile([64, 1], F32, name="y_ps", tag="str")
        nc.tensor.matmul(y_ps[:], c["w1T"][:], zv[:], start=True, stop=True)
        y1 = SBF.tile([64, 1], F32, name="y1")
        nc.vector.tensor_scalar(y1[:], y_ps[:], 0.2, None, ALU.mult)
        y_s = SBF.tile([64, 1], F32, name="y_s")
        nc.vector.tensor_tensor(y_s[:], y_ps[:], y1[:], ALU.max)
        wat_ps = PSF.tile([T - 1, 1], F32, name="wat_ps", tag="str")
        nc.tensor.matmul(wat_ps[:], c["w2T"][:], y_s[:], start=True, stop=True)
        wat_s = SBF.tile([T - 1, 1], F32, name="wat_s")
        nc.vector.tensor_copy(wat_s[:], wat_ps[:])
        watT_ps = PSF.tile([1, T - 1], F32, name="watT_ps", tag="str")
        nc.tensor.transpose(watT_ps[:], wat_s[:], c["identF"][0:7, 0:7])
        nmw = SBF.tile([1, 1], F32, name="nmw")
        nc.vector.tensor_reduce(nmw[:], watT_ps[:], AX.X, ALU.max, negate=True)
        den = SBF.tile([1, 1], F32, name="den")
        exw = SBF.tile([1, T - 1], F32, name="exw")
        nc.scalar.activation(exw[:], watT_ps[:], AF.Exp, bias=nmw[:],
                             accum_out=den[:])
        rw = SBF.tile([1, 1], F32, name="rw")
        nc.vector.reciprocal(rw[:], den[:])
        wsm = SBF.tile([1, T - 1], F32, name="wsm")
        nc.vector.tensor_scalar(wsm[:], exw[:], rw[:], None, ALU.mult)
        # broadcast wsm across 128 partitions via a K=1 matmul
        wbc_ps = PSF.tile([128, T - 1], F32, name="wbc_ps", tag="str")
        nc.tensor.matmul(wbc_ps[:], c["ones_row"][:], wsm[:],
                         start=True, stop=True)
        wbc = SBF.tile([128, T - 1], F32, name="wbc")
        nc.vector.tensor_copy(wbc[:], wbc_ps[:])

        # xx1 = w0*(x2[1]-x2[0]) + w2*(x2[3]-x2[2])
        x2 = []
        for t_ in range(4):
            xt_ = SBF.tile([64, NP], F32, name=f"x2_{t_}")
            nc.sync.dma_start(xt_[:], cc2_out[t_, 0:64, :])
            x2.append(xt_)
        d0 = SBF.tile([64, NP], F32, name="d0")
        nc.vector.tensor_tensor(d0[:], x2[1][:], x2[0][:], ALU.subtract)
        d2 = SBF.tile([64, NP], F32, name="d2")
        nc.vector.tensor_tensor(d2[:], x2[3][:], x2[2][:], ALU.subtract)
        m0 = SBF.tile([64, NP], F32, name="m0")
        nc.vector.tensor_scalar(m0[:], d0[:], wbc[0:64, 0:1], None, ALU.mult)
        nc.vector.scalar_tensor_tensor(combT2[64:128, :], d2[:],
                                       wbc[0:64, 2:3], m0[:],
                                       ALU.mult, ALU.add)
        # output head: res[n] = leaky(Wl . comb[:, n] + bl)
        res_ps = PSF.tile([128, NCH], F32, name="res_ps", tag="str")
        for k in range(NCH):
            nc.tensor.matmul(res_ps[:, k:k + 1],
                             combT2[:, k * 128:(k + 1) * 128], c["Wl"][:],
                             start=True, stop=True)
        r1 = SBF.tile([128, NCH], F32, name="r1")
        nc.vector.tensor_scalar(r1[:], res_ps[:], c["bl_rep"][:], 0.2,
                                ALU.add, ALU.mult)
        r2 = SBF.tile([128, NCH], F32, name="r2")
        nc.vector.tensor_scalar(r2[:], res_ps[:], c["bl_rep"][:], None,
                                ALU.add)
        res_s = SBF.tile([128, NCH], F32, name="res_s")
        nc.vector.tensor_tensor(res_s[:], r2[:], r1[:], ALU.max)
        nc.sync.dma_start(
            out_ap[0:1024, 0:1].rearrange("(k p) o -> p k o", p=128),
            res_s[:, 0:8].unsqueeze(2))
        nc.sync.dma_start(out_ap[1024:1026, 0:1], res_s[0:2, 8:9])

    stack.close()


# --------------------------------------------------------------------------
# entry points
# --------------------------------------------------------------------------

def _make_nc():
    if "nc" in _NC_CACHE:
        return _NC_CACHE["nc"]
    import concourse.bacc as bacc
    import concourse.mybir as mybir
    from concourse import tile

    nc = bacc.Bacc("TRN2", target_bir_lowering=False, debug=False,
                   enable_asserts=True, num_devices=NCORES)
    A = {}
    dtmap = {"f32": mybir.dt.float32, "bf16": mybir.dt.bfloat16}
    for nm, shape, dt_ in _IN_SPECS:
        A[nm] = nc.dram_tensor(
            nm, list(shape), dtmap[dt_], kind="ExternalInput").ap()
    out_h = nc.dram_tensor("out", [N, 1], mybir.dt.float32,
                           kind="ExternalOutput")
    with tile.TileContext(nc) as tc:
        build_program(tc, A, out_h.ap())
    nc.compile()
    _NC_CACHE["nc"] = nc
    return nc


def kernel(**inputs):
    from concourse.bass_utils import run_bass_kernel_spmd
    nc = _make_nc()
    in_maps = _host_prep(inputs)
    res = run_bass_kernel_spmd(nc, in_maps, list(range(NCORES)))
    return np.asarray(res.results[0]["out"])


# revision 29
# speedup vs baseline: 1.3047x; 1.0288x over previous
"""HGAT (GRU + decayed attention + 2x HypergraphConv over 9 hypergraphs) on 8 trn2 cores.

Strategy:
  - Host: densify each hypergraph incidence list into dense [1152,1152]
    operators holding RAW integer counts (exact in bf16), shipped in both
    layouts (node-major / edge-major).  B^-1 / D^-1 / bias scalings are folded
    into per-column vector ops / rank-1 matmuls on device.  The attention
    decay factors exp(-ab*delta) are precomputed on host.
  - Device (SPMD, 8 cores): GRU+attention sharded over nodes (144/core) with
    gates accumulated directly in PSUM (bf16 matmuls), AllGather (bf16) the
    attention output, data-parallel hypergraph convs (core c: timestep c)
    with column-tiled matmul pairs (2 concurrent 64-wide output groups),
    AllGather timestep conv results + per-timestep sums (f32), the
    global-hyp conv computed redundantly on all cores DURING the second
    AllGather, final temporal attention + output head with a PE-broadcast of
    softmax weights (no DRAM bounce).
"""
import numpy as np
import ml_dtypes

N, T, H, F_IN, E = 1026, 8, 64, 5, 1026
NP = 1152            # padded N and E (9 * 128)
NCORES = 8
SL = NP // NCORES    # 144 nodes per core
NCH = NP // 128      # 9 contraction chunks
BF = ml_dtypes.bfloat16

_NC_CACHE = {}


# --------------------------------------------------------------------------
# host-side prep
# --------------------------------------------------------------------------

def _densify(idx):
    node = idx[0].astype(np.int64)
    edge = idx[1].astype(np.int64)
    Hm = np.bincount(node * NP + edge, minlength=N * NP).reshape(N, NP)
    Hp = np.zeros((NP, NP), np.float32)
    Hp[:N] = Hm.astype(np.float32)
    degn = Hp.sum(1)
    dege = Hp.sum(0)
    Dinv = np.where(degn > 0, 1.0 / degn, 0.0).astype(np.float32)
    Binv = np.where(dege > 0, 1.0 / dege, 0.0).astype(np.float32)
    Hn = np.ascontiguousarray(Hp.astype(BF))                  # [n, e] raw counts
    HTe = np.ascontiguousarray(Hp.T.astype(BF))               # [e, n] raw counts
    return Hn, HTe, degn, Dinv, Binv


def _host_prep(inp):
    f32 = np.float32
    price = np.asarray(inp["price_input"], f32)          # [N, T, F]
    hyp_T = np.asarray(inp["hyp_T"])                     # [T, 2, nnz]
    hyp = np.asarray(inp["hyp"])                         # [2, nnz]

    WihT = np.ascontiguousarray(np.asarray(inp["Wih"], f32).T)   # [5, 192]
    WhhT = np.ascontiguousarray(np.asarray(inp["Whh"], f32).T)   # [64, 192]
    bih = np.asarray(inp["bih"], f32)
    bhh = np.asarray(inp["bhh"], f32)

    shared = {
        "WihT_rz": np.ascontiguousarray(WihT[:, 0:128]).astype(BF),
        "WihT_n": np.ascontiguousarray(WihT[:, 128:192]).astype(BF),
        "WhhT_rz": np.ascontiguousarray(WhhT[:, 0:128]).astype(BF),
        "WhhT_n": np.ascontiguousarray(WhhT[:, 128:192]).astype(BF),
        "b_rzsum": np.ascontiguousarray((bih[0:128] + bhh[0:128])[:, None]),
        "bih_n": np.ascontiguousarray(bih[128:192, None]),
        "bhh_n": np.ascontiguousarray(bhh[128:192, None]),
        "bhh_n_pad": np.ascontiguousarray(
            np.concatenate([np.zeros(64, f32), bhh[128:192]])[:, None]),
        "Win": np.asarray(inp["Win"], BF),
        "Wout": np.asarray(inp["Wout"], BF),
        "theta1": np.asarray(inp["theta1"], BF),
        "theta2": np.asarray(inp["theta2"], BF),
        "b1_col": np.ascontiguousarray(np.asarray(inp["bias1"], f32)[:, None]),
        "b2_col": np.ascontiguousarray(np.asarray(inp["bias2"], f32)[:, None]),
        "w1T": np.ascontiguousarray(np.asarray(inp["w1"], f32).T),   # [7, 64]
        "w2T": np.ascontiguousarray(np.asarray(inp["w2"], f32).T),   # [64, 7]
        "Wl": np.asarray(inp["Wl"], BF),                            # [128, 1]
        "bl_rep": np.full((128, 1), np.asarray(inp["bl"], f32)[0], f32),
        "ones_row": np.ones((1, 128), f32),
        "identF": np.eye(128, dtype=f32),
        "identB": np.eye(128, dtype=BF),
    }

    HnG, HTeG, degG, DinvG, BinvG = _densify(hyp)
    shared["Hn_G"] = HnG
    shared["HTe_G"] = HTeG
    shared["Binv_G"] = np.ascontiguousarray(BinvG[None, :])
    shared["Dinv_G"] = np.ascontiguousarray(DinvG[None, :])

    price_p = np.zeros((NP, T, F_IN), f32)
    price_p[:N] = price
    ae_p = np.zeros((NP,), f32)
    ae_p[:N] = np.asarray(inp["ae"], f32)[:, 0, 0]
    ab_p = np.zeros((NP,), f32)
    ab_p[:N] = np.asarray(inp["ab"], f32)[:, 0, 0]
    delta = np.arange(T - 1, -1, -1, dtype=f32)
    bt_full = np.exp(-ab_p[:, None] * delta[None, :])    # [NP, T]

    in_maps = []
    for c in range(NCORES):
        sl = slice(c * SL, (c + 1) * SL)
        m = dict(shared)
        m["x5"] = np.ascontiguousarray(
            price_p[sl].transpose(2, 1, 0).reshape(F_IN, T * SL)).astype(BF)
        m["ae_col"] = np.ascontiguousarray(ae_p[sl, None])
        m["bt_sl"] = np.ascontiguousarray(bt_full[sl])
        HnL, HTeL, degL, DinvL, BinvL = _densify(hyp_T[c])
        m["Hn_L"] = HnL
        m["HTe_L"] = HTeL
        m["Binv_L"] = np.ascontiguousarray(BinvL[None, :])
        m["Dinv_L"] = np.ascontiguousarray(DinvL[None, :])
        in_maps.append(m)
    return in_maps


_IN_SPECS = [
    ("x5", (F_IN, NP), "bf16"),
    ("WihT_rz", (F_IN, 128), "bf16"), ("WihT_n", (F_IN, 64), "bf16"),
    ("WhhT_rz", (64, 128), "bf16"), ("WhhT_n", (64, 64), "bf16"),
    ("b_rzsum", (128, 1), "f32"),
    ("bih_n", (64, 1), "f32"), ("bhh_n", (64, 1), "f32"),
    ("bhh_n_pad", (128, 1), "f32"),
    ("identB", (128, 128), "bf16"),
    ("Hn_L", (NP, NP), "bf16"), ("HTe_L", (NP, NP), "bf16"),
    ("Hn_G", (NP, NP), "bf16"), ("HTe_G", (NP, NP), "bf16"),
    ("Win", (64, 64), "bf16"), ("Wout", (128, 64), "bf16"),
    ("ae_col", (SL, 1), "f32"), ("bt_sl", (SL, T), "f32"),
    ("identF", (128, 128), "f32"),
    ("theta1", (64, 64), "bf16"), ("theta2", (64, 64), "bf16"),
    ("b1_col", (64, 1), "f32"), ("b2_col", (64, 1), "f32"),
    ("w1T", (T - 1, 64), "f32"), ("w2T", (64, T - 1), "f32"),
    ("Wl", (128, 1), "bf16"), ("bl_rep", (128, 1), "f32"),
    ("ones_row", (1, 128), "f32"),
    ("Binv_L", (1, NP), "f32"), ("Binv_G", (1, NP), "f32"),
    ("Dinv_L", (1, NP), "f32"), ("Dinv_G", (1, NP), "f32"),
]

# DMA issue order: GRU-critical first, then H operators, then attention
# consts, then conv/final consts (single in-order DMA queue).
_LOAD_ORDER = [
    "x5", "WihT_rz", "WihT_n", "WhhT_rz", "WhhT_n", "b_rzsum", "bih_n",
    "bhh_n", "bhh_n_pad", "identB",
    None,  # marker: H matrices here
    "Win", "Wout", "identF",
    "theta1", "theta2", "b1_col", "b2_col", "w1T", "w2T", "Wl", "bl_rep",
    "ones_row",
]


# --------------------------------------------------------------------------
# device program
# --------------------------------------------------------------------------

def build_program(tc, A, out_ap):
    """Emit the SPMD program. A: dict name -> dram AP. out_ap: [1026,1] f32."""
    import contextlib
    import concourse.bass as bass
    import concourse.mybir as mybir

    nc = tc.nc
    F32 = mybir.dt.float32
    BF16 = mybir.dt.bfloat16
    AF = mybir.ActivationFunctionType
    ALU = mybir.AluOpType
    AX = mybir.AxisListType
    CH3 = ((0, 512), (512, 512), (1024, 128))
    groups = [list(range(NCORES))]

    stack = contextlib.ExitStack()
    CP = stack.enter_context(tc.tile_pool(name="consts", bufs=1))
    WK = stack.enter_context(tc.tile_pool(name="work", bufs=1))
    HP = stack.enter_context(tc.tile_pool(name="hmat", bufs=1))
    DR = stack.enter_context(tc.tile_pool(name="dram", bufs=1, space="DRAM"))

    def load(pool, name, shape, dtype, src_ap):
        t = pool.tile(shape, dtype, name=name)
        nc.sync.dma_start(t[:], src_ap)
        return t

    spec_by_name = dict((s[0], s) for s in _IN_SPECS)
    dtmap = {"f32": F32, "bf16": BF16}
    c = {}
    Hmats = {}
    for nm in _LOAD_ORDER:
        if nm is None:
            for hn in ("Hn_L", "HTe_L", "Hn_G", "HTe_G"):
                tiles = []
                for k in range(NCH):
                    tiles.append(load(HP, f"{hn}_{k}", [128, NP], BF16,
                                      A[hn][k * 128:(k + 1) * 128, :]))
                Hmats[hn] = tiles
            continue
        spec = spec_by_name[nm]
        c[nm] = load(CP, f"c_{nm}", list(spec[1]), dtmap[spec[2]], A[nm][:])

    aeA = load(CP, "aeA", [128, 1], F32, A["ae_col"][0:128])
    aeB = load(CP, "aeB", [16, 1], F32, A["ae_col"][128:SL])
    btA = load(CP, "btA", [128, T], F32, A["bt_sl"][0:128, :])
    btB = load(CP, "btB", [16, T], F32, A["bt_sl"][128:SL, :])

    # broadcast rows -> [64, NP] tiles (partition-broadcast via DMA, last)
    bcast = {}
    for nm, dt_ in (("Binv_L", F32), ("Binv_G", F32),
                    ("Dinv_L", F32), ("Dinv_G", F32)):
        t = CP.tile([64, NP], dt_, name=f"bc_{nm}")
        nc.sync.dma_start(t[:], A[nm][0:1, :].broadcast_to([64, NP]))
        bcast[nm] = t

    identF64 = c["identF"][0:64, 0:64]
    identB64 = c["identB"][0:64, 0:64]

    # ---- persistent work tiles ----
    ctxT = WK.tile([64, T * SL], BF16, name="ctxT")         # [h, (t n)]
    ctx_nA = WK.tile([128, T, 64], BF16, name="ctx_nA")
    ctx_nB = WK.tile([16, T, 64], BF16, name="ctx_nB")
    outT_full = WK.tile([64, NP], BF16, name="outT_full")   # gathered attention out
    x1T = WK.tile([64, NP], BF16, name="x1T")               # L1 out (Dinv deferred)
    x1gT = WK.tile([64, NP], BF16, name="x1gT")             # G1 out (Dinv deferred)
    pay = WK.tile([65, NP], F32, name="pay")                # x2 + S row
    combT2 = WK.tile([128, NP], BF16, name="combT2")         # [xgT ; xx1T]

    # ======================= GRU =======================
    with tc.tile_pool(name="sb_gi", bufs=1) as SBGI:
        gi_n = SBGI.tile([64, T * SL], F32, name="gi_n")
        with tc.tile_pool(name="ps_gi", bufs=1, space="PSUM") as PSGI:
            gi_n_ps = PSGI.tile([64, T * SL], F32, name="gi_n_ps", tag="gi")
            for o, w in CH3:
                nc.tensor.matmul(gi_n_ps[:, o:o + w], c["WihT_n"][:],
                                 c["x5"][:, o:o + w], start=True, stop=True)
            nc.scalar.activation(gi_n[:], gi_n_ps[:], AF.Identity,
                                 bias=c["bih_n"][:])

        with tc.tile_pool(name="ps_rz", bufs=3, space="PSUM") as PSR, \
             tc.tile_pool(name="ps_gru", bufs=1, space="PSUM") as PSG, \
             tc.tile_pool(name="sb_gru", bufs=2) as SBG:
            for t in range(T):
                s = slice(t * SL, (t + 1) * SL)
                sp = slice((t - 1) * SL, t * SL)
                rz = SBG.tile([128, SL], F32, name="rz", tag="rz")
                z0 = SBG.tile([64, SL], F32, name="z0", tag="z0")
                wn = SBG.tile([64, SL], F32, name="wn", tag="wn")
                un = SBG.tile([64, SL], F32, name="un", tag="un")
                nt = SBG.tile([64, SL], F32, name="nt", tag="nt")
                mt = SBG.tile([64, SL], F32, name="mt", tag="mt")
                # gates rz accumulated in PSUM: Wih part first (independent
                # of the recurrence), then the Whh part joins the group
                g_rz = PSR.tile([128, SL], F32, name="g_rz", tag="psrz")
                nc.tensor.matmul(g_rz[:], c["WihT_rz"][:], c["x5"][:, s],
                                 start=True, stop=(t == 0))
                if t == 0:
                    nc.scalar.activation(rz[:], g_rz[:], AF.Sigmoid,
                                         bias=c["b_rzsum"][:])
                    nc.scalar.activation(z0[:], rz[64:128, :], AF.Copy)
                    nc.vector.tensor_scalar(wn[:], rz[0:64, :], c["bhh_n"][:],
                                            None, ALU.mult)
                    nc.vector.tensor_tensor(un[:], gi_n[:, s], wn[:], ALU.add)
                    nc.scalar.activation(nt[:], un[:], AF.Tanh)
                    nc.vector.tensor_tensor(mt[:], nt[:], z0[:], ALU.mult)
                    nc.vector.tensor_tensor(ctxT[:, s], nt[:], mt[:],
                                            ALU.subtract)
                else:
                    nc.tensor.matmul(g_rz[:], c["WhhT_rz"][:],
                                     ctxT[:, sp], start=False, stop=True)
                    gh_n = PSG.tile([64, SL], F32, name="gh_n", tag="gh_n")
                    nc.tensor.matmul(gh_n[:], c["WhhT_n"][:], ctxT[:, sp],
                                     start=True, stop=True)
                    nc.scalar.activation(rz[:], g_rz[:], AF.Sigmoid,
                                         bias=c["b_rzsum"][:])
                    nc.scalar.activation(z0[:], rz[64:128, :], AF.Copy)
                    nc.vector.scalar_tensor_tensor(wn[:], gh_n[:],
                                                   c["bhh_n"][:],
                                                   rz[0:64, :], ALU.add,
                                                   ALU.mult)
                    nc.vector.tensor_tensor(un[:], gi_n[:, s], wn[:], ALU.add)
                    nc.scalar.activation(nt[:], un[:], AF.Tanh)
                    dt_ = SBG.tile([64, SL], F32, name="dt_", tag="dt_")
                    nc.vector.tensor_tensor(dt_[:], ctxT[:, sp], nt[:],
                                            ALU.subtract)
                    nc.vector.tensor_tensor(mt[:], dt_[:], z0[:], ALU.mult)
                    nc.vector.tensor_tensor(ctxT[:, s], mt[:], nt[:], ALU.add)
                # node-major ctx for attention via PE transposes
                trA = PSG.tile([128, 64], BF16, name="trA", tag="trA")
                nc.tensor.transpose(trA[:], ctxT[:, t * SL:t * SL + 128],
                                    identB64)
                nc.vector.tensor_copy(ctx_nA[:, t, :], trA[:])
                trB = PSG.tile([16, 64], BF16, name="trB", tag="trB")
                nc.tensor.transpose(trB[:], ctxT[:, t * SL + 128:(t + 1) * SL],
                                    identB64)
                nc.vector.tensor_copy(ctx_nB[:, t, :], trB[:])

    # ======================= attention =======================
    with tc.tile_pool(name="ps_att", bufs=1, space="PSUM") as PSA, \
         tc.tile_pool(name="sb_att", bufs=1) as SBA:
        lastT = ctxT[:, 7 * SL:8 * SL]
        qT_ps = PSA.tile([64, SL], F32, name="qT_ps", tag="qT")
        nc.tensor.matmul(qT_ps[:], c["Win"][:], lastT, start=True, stop=True)
        combT = SBA.tile([128, SL], BF16, name="combT")
        nc.scalar.activation(combT[64:128, :], qT_ps[:], AF.Copy)

        for nm, np_, ctx_n, ae_t, bt_sl, csl in (
                ("A", 128, ctx_nA, aeA[:], btA[:], slice(0, 128)),
                ("B", 16, ctx_nB, aeB[:], btB[:], slice(128, SL))):
            q_ps = PSA.tile([np_, 64], F32, name=f"q_ps{nm}", tag=f"q{nm}")
            nc.tensor.matmul(q_ps[:], lastT[:, csl], c["Win"][:],
                             start=True, stop=True)
            q_s = SBA.tile([np_, 64], F32, name=f"q_s{nm}")
            nc.scalar.activation(q_s[:], q_ps[:], AF.Copy)
            prod = SBA.tile([np_, T, 64], F32, name=f"prod{nm}")
            nc.vector.tensor_tensor(
                prod[:], ctx_n[:],
                q_s[:].unsqueeze(1).broadcast_to([np_, T, 64]), ALU.mult)
            sc = SBA.tile([np_, T], F32, name=f"sc{nm}")
            nc.vector.tensor_reduce(sc[:], prod[:], AX.X, ALU.add)
            den = SBA.tile([np_, 1], F32, name=f"den{nm}")
            ex = SBA.tile([np_, T], F32, name=f"ex{nm}")
            nc.scalar.activation(ex[:], sc[:], AF.Exp, accum_out=den[:])
            rcp = SBA.tile([np_, 1], F32, name=f"rcp{nm}")
            nc.vector.reciprocal(rcp[:], den[:])
            wA = SBA.tile([np_, T], F32, name=f"wA{nm}")
            nc.vector.tensor_scalar(wA[:], ex[:], rcp[:], None, ALU.mult)
            P_t = SBA.tile([np_, T, 64], F32, name=f"P_t{nm}")
            nc.vector.tensor_tensor(
                P_t[:], ctx_n[:],
                wA[:].unsqueeze(2).broadcast_to([np_, T, 64]), ALU.mult)
            G_t = SBA.tile([np_, T, 64], F32, name=f"G_t{nm}")
            nc.vector.tensor_tensor(
                G_t[:], P_t[:],
                bt_sl.unsqueeze(2).broadcast_to([np_, T, 64]), ALU.mult)
            t2_t = SBA.tile([np_, T, 64], F32, name=f"t2_t{nm}")
            nc.scalar.activation(t2_t[:], G_t[:], AF.Relu, scale=ae_t)
            sm = SBA.tile([np_, T, 64], F32, name=f"sm{nm}")
            nc.vector.tensor_tensor(sm[:], P_t[:], t2_t[:], ALU.add)
            mixs = SBA.tile([np_, 64], F32, name=f"mixs{nm}")
            nc.vector.tensor_reduce(
                mixs[:], sm[:].rearrange("p t h -> p h t"), AX.X, ALU.add)
            # transpose mixs into combT rows 0:64
            mtr = PSA.tile([64, np_], F32, name=f"mtr{nm}", tag=f"mtr{nm}")
            nc.tensor.transpose(mtr[:], mixs[:], c["identF"][0:np_, 0:np_])
            nc.scalar.activation(combT[0:64, csl], mtr[:], AF.Copy)

        outT_ps = PSA.tile([64, SL], F32, name="outT_ps", tag="outT")
        nc.tensor.matmul(outT_ps[:], c["Wout"][:], combT[:],
                         start=True, stop=True)
        outT_slice = SBA.tile([64, SL], BF16, name="outT_slice")
        nc.scalar.activation(outT_slice[:], outT_ps[:], AF.Tanh)

        # ---- collective 1: allgather attention output (bf16) ----
        cc1_in = DR.tile([64, SL], BF16, name="cc1_in")
        cc1_out = DR.tile([NCORES, 64, SL], BF16, name="cc1_out",
                          addr_space="Shared")
        nc.sync.dma_start(cc1_in[:], outT_slice[:])
        nc.gpsimd.collective_compute(
            "AllGather", ALU.bypass, replica_groups=groups,
            ins=[cc1_in[:].opt()], outs=[cc1_out[:].opt()])
        nc.sync.dma_start(
            outT_full[:].rearrange("p (c n) -> p c n", c=NCORES),
            cc1_out[:].rearrange("c p n -> p c n"))

    # ======================= hypergraph convs =======================
    conv_stack = contextlib.ExitStack()
    PSX = conv_stack.enter_context(tc.tile_pool(name="ps_xp", bufs=1, space="PSUM"))
    PAcc = conv_stack.enter_context(tc.tile_pool(name="ps_acc", bufs=1, space="PSUM"))
    SBC = conv_stack.enter_context(tc.tile_pool(name="sb_conv", bufs=1))
    SBX = conv_stack.enter_context(tc.tile_pool(name="sb_xp", bufs=2))

    EVEN = [k for k in range(NCH) if k % 2 == 0]
    ODD = [k for k in range(NCH) if k % 2 == 1]

    def cb_front(xT_in, theta_t, Hn_ts, Binv_bc, tag, acc):
        """xp = theta^T x; e^T = B^-1 (xp^T Hn); edge-major bf16 chunks."""
        xp_ps = PSX.tile([128, NCH * 64], F32, name=f"xp_{tag}", tag="xp")
        for k in range(NCH):
            nc.tensor.matmul(xp_ps[:, k * 64:(k + 1) * 64],
                             xT_in[:, k * 128:(k + 1) * 128], theta_t[:],
                             start=True, stop=True)
        xpbf = SBX.tile([128, NCH, 64], BF16, name=f"xpbf_{tag}", tag="xpbf")
        nc.scalar.activation(
            xpbf[:], xp_ps[:].rearrange("p (k h) -> p k h", k=NCH), AF.Copy)
        eb_ps = PAcc.tile([128, NP], F32, name=f"ebT_{tag}", tag=acc)
        for i in range(len(EVEN)):
            for o, w in CH3:
                k = EVEN[i]
                nc.tensor.matmul(eb_ps[0:64, o:o + w], xpbf[:, k, :],
                                 Hn_ts[k][:, o:o + w],
                                 start=(k == EVEN[0]), stop=(k == EVEN[-1]))
                if i < len(ODD):
                    k = ODD[i]
                    nc.tensor.matmul(eb_ps[64:128, o:o + w], xpbf[:, k, :],
                                     Hn_ts[k][:, o:o + w],
                                     start=(k == ODD[0]), stop=(k == ODD[-1]))
        e_top = SBC.tile([64, NP], BF16, name=f"etop_{tag}", tag="etop")
        nc.scalar.activation(e_top[:], eb_ps[0:64, :], AF.Copy)
        e_sum = SBC.tile([64, NP], F32, name=f"esum_{tag}", tag="esum")
        nc.vector.tensor_tensor(e_sum[:], e_top[:], eb_ps[64:128, :], ALU.add)
        ebTbf = SBC.tile([64, NP], BF16, name=f"ebTbf_{tag}", tag="ebTbf")
        nc.vector.tensor_tensor(ebTbf[:], e_sum[:], Binv_bc[:], ALU.mult)
        tr_ps = PSX.tile([128, NCH * 64], BF16, name=f"tr_{tag}", tag="xp")
        for k in range(NCH):
            nc.tensor.transpose(tr_ps[:, k * 64:(k + 1) * 64],
                                ebTbf[:, k * 128:(k + 1) * 128], identB64)
        ebbf = SBC.tile([128, NCH, 64], BF16, name=f"ebbf_{tag}", tag=f"eb{tag}")
        nc.scalar.activation(
            ebbf[:], tr_ps[:].rearrange("p (k h) -> p k h", k=NCH), AF.Copy)
        return ebbf

    def cb_back(ebbf, HTe_ts, b_col, Dinv_bc, tag, acc, out_dst, S_col=None):
        """out^T = leaky(D^-1 (e^T HTe) + b)."""
        oT_ps = PAcc.tile([128, NP], F32, name=f"oT_{tag}", tag=acc)
        for i in range(len(EVEN)):
            for o, w in CH3:
                k = EVEN[i]
                nc.tensor.matmul(oT_ps[0:64, o:o + w], ebbf[:, k, :],
                                 HTe_ts[k][:, o:o + w],
                                 start=(k == EVEN[0]), stop=(k == EVEN[-1]))
                if i < len(ODD):
                    k = ODD[i]
                    nc.tensor.matmul(oT_ps[64:128, o:o + w], ebbf[:, k, :],
                                     HTe_ts[k][:, o:o + w],
                                     start=(k == ODD[0]), stop=(k == ODD[-1]))
        o_top = SBC.tile([64, NP], BF16, name=f"otop_{tag}", tag="otop")
        nc.scalar.activation(o_top[:], oT_ps[0:64, :], AF.Copy)
        u = SBC.tile([64, NP], F32, name=f"u_{tag}", tag="u")
        nc.vector.tensor_tensor(u[:], o_top[:], oT_ps[64:128, :], ALU.add)
        m = SBC.tile([64, NP], F32, name=f"m_{tag}", tag="m")
        nc.vector.tensor_tensor(m[:], u[:], Dinv_bc[:], ALU.mult)
        l1 = SBC.tile([64, NP], F32, name=f"l1_{tag}", tag="lk1")
        nc.vector.tensor_scalar(l1[:], m[:], b_col[:], 0.2, ALU.add, ALU.mult)
        l2 = SBC.tile([64, NP], F32, name=f"l2_{tag}", tag="lk2")
        nc.vector.tensor_scalar(l2[:], m[:], b_col[:], None, ALU.add)
        if S_col is not None:
            nc.vector.scalar_tensor_tensor(out_dst, l2[:], 1.0, l1[:],
                                           ALU.mult, ALU.max,
                                           accum_out=S_col)
        else:
            nc.vector.tensor_tensor(out_dst, l2[:], l1[:], ALU.max)

    S_col = SBC.tile([64, 1], F32, name="S_col")

    # interleaved L/G conv emission: G fills PE bubbles of L epilogues;
    # G2's back half runs under the second AllGather.
    ebbfL1 = cb_front(outT_full[:], c["theta1"], Hmats["Hn_L"],
                      bcast["Binv_L"], "L1", "accL")
    ebbfG1 = cb_front(outT_full[:], c["theta1"], Hmats["Hn_G"],
                      bcast["Binv_G"], "G1", "accG")
    cb_back(ebbfL1, Hmats["HTe_L"], c["b1_col"], bcast["Dinv_L"], "L1",
            "accL", x1T[:])
    cb_back(ebbfG1, Hmats["HTe_G"], c["b1_col"], bcast["Dinv_G"], "G1",
            "accG", x1gT[:])
    ebbfL2 = cb_front(x1T[:], c["theta2"], Hmats["Hn_L"],
                      bcast["Binv_L"], "L2", "accL")
    ebbfG2 = cb_front(x1gT[:], c["theta2"], Hmats["Hn_G"],
                      bcast["Binv_G"], "G2", "accG")
    cb_back(ebbfL2, Hmats["HTe_L"], c["b2_col"], bcast["Dinv_L"], "L2",
            "accL", pay[0:64, :], S_col=S_col)

    # S scalar into pay row 64
    nc.vector.memset(pay[64:65, :], 0.0)
    S_tr = PSX.tile([1, 64], F32, name="S_tr", tag="xp")
    nc.tensor.transpose(S_tr[:], S_col[:], identF64)
    nc.vector.tensor_reduce(pay[64:65, 0:1], S_tr[:], AX.X, ALU.add)

    # ---- collective 2: allgather conv results + sums (f32) ----
    cc2_in = DR.tile([65, NP], F32, name="cc2_in")
    cc2_out = DR.tile([NCORES, 65, NP], F32, name="cc2_out",
                      addr_space="Shared")
    nc.sync.dma_start(cc2_in[:], pay[:])
    nc.gpsimd.collective_compute(
        "AllGather", ALU.bypass, replica_groups=groups,
        ins=[cc2_in[:].opt()], outs=[cc2_out[:].opt()])

    # global conv layer-2 back half overlaps the collective
    cb_back(ebbfG2, Hmats["HTe_G"], c["b2_col"], bcast["Dinv_G"], "G2",
            "accG", combT2[0:64, :])

    conv_stack.close()

    # ======================= final stage =======================
    with tc.tile_pool(name="sb_fin", bufs=1) as SBF, \
         tc.tile_pool(name="ps_fin", bufs=1, space="PSUM") as PSF:
        # temporal attention weights from the gathered S values
        Sg0 = SBF.tile([T - 1, 1], F32, name="Sg0")
        nc.sync.dma_start(Sg0[:], cc2_out[0:T - 1, 64, 0:1])
        Sg1 = SBF.tile([T - 1, 1], F32, name="Sg1")
        nc.sync.dma_start(Sg1[:], cc2_out[1:T, 64, 0:1])
        zv = SBF.tile([T - 1, 1], F32, name="zv")
        nc.vector.tensor_tensor(zv[:], Sg1[:], Sg0[:], ALU.subtract)
        y_ps = PSF.tile([64, 1], F32, name="y_ps", tag="str")
        nc.tensor.matmul(y_ps[:], c["w1T"][:], zv[:], start=True, stop=True)
        y1 = SBF.tile([64, 1], F32, name="y1")
        nc.vector.tensor_scalar(y1[:], y_ps[:], 0.2, None, ALU.mult)
        y_s = SBF.tile([64, 1], F32, name="y_s")
        nc.vector.tensor_tensor(y_s[:], y_ps[:], y1[:], ALU.max)
        wat_ps = PSF.tile([T - 1, 1], F32, name="wat_ps", tag="str")
        nc.tensor.matmul(wat_ps[:], c["w2T"][:], y_s[:], start=True, stop=True)
        wat_s = SBF.tile([T - 1, 1], F32, name="wat_s")
        nc.vector.tensor_copy(wat_s[:], wat_ps[:])
        watT_ps = PSF.tile([1, T - 1], F32, name="watT_ps", tag="str")
        nc.tensor.transpose(watT_ps[:], wat_s[:], c["identF"][0:7, 0:7])
        nmw = SBF.tile([1, 1], F32, name="nmw")
        nc.vector.tensor_reduce(nmw[:], watT_ps[:], AX.X, ALU.max, negate=True)
        den = SBF.tile([1, 1], F32, name="den")
        exw = SBF.tile([1, T - 1], F32, name="exw")
        nc.scalar.activation(exw[:], watT_ps[:], AF.Exp, bias=nmw[:],
                             accum_out=den[:])
        rw = SBF.tile([1, 1], F32, name="rw")
        nc.vector.reciprocal(rw[:], den[:])
        wsm = SBF.tile([1, T - 1], F32, name="wsm")
        nc.vector.tensor_scalar(wsm[:], exw[:], rw[:], None, ALU.mult)
        # broadcast wsm across 128 partitions via a K=1 matmul
        wbc_ps = PSF.tile([128, T - 1], F32, name="wbc_ps", tag="str")
        nc.tensor.matmul(wbc_ps[:], c["ones_row"][:], wsm[:],
                         start=True, stop=True)
        wbc = SBF.tile([128, T - 1], F32, name="wbc")
        nc.vector.tensor_copy(wbc[:], wbc_ps[:])

        # xx1 = w0*(x2[1]-x2[0]) + w2*(x2[3]-x2[2])
        x2 = []
        for t_ in range(4):
            xt_ = SBF.tile([64, NP], F32, name=f"x2_{t_}")
            nc.sync.dma_start(xt_[:], cc2_out[t_, 0:64, :])
            x2.append(xt_)
        d0 = SBF.tile([64, NP], F32, name="d0")
        nc.vector.tensor_tensor(d0[:], x2[1][:], x2[0][:], ALU.subtract)
        d2 = SBF.tile([64, NP], F32, name="d2")
        nc.vector.tensor_tensor(d2[:], x2[3][:], x2[2][:], ALU.subtract)
        m0 = SBF.tile([64, NP], F32, name="m0")
        nc.vector.tensor_scalar(m0[:], d0[:], wbc[0:64, 0:1], None, ALU.mult)
        nc.vector.scalar_tensor_tensor(combT2[64:128, :], d2[:],
                                       wbc[0:64, 2:3], m0[:],
                                       ALU.mult, ALU.add)
        # output head: res[n] = leaky(Wl . comb[:, n] + bl)
        res_ps = PSF.tile([128, NCH], F32, name="res_ps", tag="str")
        for k in range(NCH):
            nc.tensor.matmul(res_ps[:, k:k + 1],
                             combT2[:, k * 128:(k + 1) * 128], c["Wl"][:],
                             start=True, stop=True)
        r1 = SBF.tile([128, NCH], F32, name="r1")
        nc.vector.tensor_scalar(r1[:], res_ps[:], c["bl_rep"][:], 0.2,
                                ALU.add, ALU.mult)
        r2 = SBF.tile([128, NCH], F32, name="r2")
        nc.vector.tensor_scalar(r2[:], res_ps[:], c["bl_rep"][:], None,
                                ALU.add)
        res_s = SBF.tile([128, NCH], F32, name="res_s")
        nc.vector.tensor_tensor(res_s[:], r2[:], r1[:], ALU.max)
        nc.sync.dma_start(
            out_ap[0:1024, 0:1].rearrange("(k p) o -> p k o", p=128),
            res_s[:, 0:8].unsqueeze(2))
        nc.sync.dma_start(out_ap[1024:1026, 0:1], res_s[0:2, 8:9])

    stack.close()


# --------------------------------------------------------------------------
# entry points
# --------------------------------------------------------------------------

def _make_nc():
    if "nc" in _NC_CACHE:
        return _NC_CACHE["nc"]
    import concourse.bacc as bacc
    import concourse.mybir as mybir
    from concourse import tile

    nc = bacc.Bacc("TRN2", target_bir_lowering=False, debug=False,
                   enable_asserts=True, num_devices=NCORES)
    A = {}
    dtmap = {"f32": mybir.dt.float32, "bf16": mybir.dt.bfloat16}
    for nm, shape, dt_ in _IN_SPECS:
        A[nm] = nc.dram_tensor(
            nm, list(shape), dtmap[dt_], kind="ExternalInput").ap()
    out_h = nc.dram_tensor("out", [N, 1], mybir.dt.float32,
                           kind="ExternalOutput")
    with tile.TileContext(nc) as tc:
        build_program(tc, A, out_h.ap())
    nc.compile()
    _NC_CACHE["nc"] = nc
    return nc


def kernel(**inputs):
    from concourse.bass_utils import run_bass_kernel_spmd
    nc = _make_nc()
    in_maps = _host_prep(inputs)
    res = run_bass_kernel_spmd(nc, in_maps, list(range(NCORES)))
    return np.asarray(res.results[0]["out"])
